# revision 1
# baseline (speedup 1.0000x reference)
"""Trainium2 Bass kernel for nn_ActorNetwork (gnn_message_passing).

Pure data-parallel across 8 NeuronCores: each core processes 8192 of the
65536 batch rows; small weights are replicated.

v2 layout: feature-major throughout, with the attention computed in a fused
(neighbor-pair, macro-column) layout that never transposes i2 back to
batch-major:
  - i1/i2 as 4 wide matmuls each (neighbor-pairs packed on partitions,
    (o,b) = 512 macro columns streamed).
  - score_n[b] = sum_d i2 * q2 via one elementwise multiply (q2 produced
    partition-duplicated for free by a duplicated-column Wq*Wk^T stationary)
    and per-pair ones-matmul partition reductions, accumulated in PSUM on
    top of the PE-transposed -1e30 mask rows -> masked scores directly.
  - softmax over an [8, 512] tile (exp on Act, denominator via ones-matmul,
    reciprocal-dup via 1x8 matmul, one multiply -> alpha).
  - alpha broadcast back to the (r,d) partition layout via 4 selector
    matmuls; weighted i2 (cmul) feeds h1 directly through a row-duplicated
    Wvc stationary -- the attention output is never materialized.
Elementwise/copy work is spread across DVE / Act / Pool; all wide moving
operands are bf16 (DVE 2x mode, PE 1 cyc/col).
"""

import os

import numpy as np
import ml_dtypes

import concourse.bass as bass
import concourse.tile as tile
from concourse import bacc
from concourse import mybir
from concourse.bass_utils import run_bass_kernel_spmd
from concourse.masks import make_identity

F32 = mybir.dt.float32
F32R = mybir.dt.float32r
BF16 = mybir.dt.bfloat16

N_CORES = 8
B_FULL = 65536
RPC = B_FULL // N_CORES        # rows per core = 8192
MACRO = 512                    # batch rows per macro tile
P = 128

Relu = mybir.ActivationFunctionType.Relu
Tanh = mybir.ActivationFunctionType.Tanh
Exp = mybir.ActivationFunctionType.Exp
Alu = mybir.AluOpType
AX = mybir.AxisListType


def build(rpc=RPC, macro=MACRO):
    nm = rpc // macro          # macro tiles per core
    nb = macro // P            # 128-row blocks per macro tile

    nc = bacc.Bacc()

    s0 = nc.declare_dram_parameter("state0", [rpc, 6], F32R, isOutput=False)
    s1 = nc.declare_dram_parameter("state1", [rpc, 1024], F32, isOutput=False)
    s2 = nc.declare_dram_parameter("state2", [rpc, 8, 7], F32, isOutput=False)
    wg_d = nc.declare_dram_parameter("wg_bf", [1024, 64], BF16, isOutput=False)
    ws1_d = nc.declare_dram_parameter("ws1_blk", [56, 4, 128], BF16, isOutput=False)
    ws2_d = nc.declare_dram_parameter("ws2_blk", [128, 128], BF16, isOutput=False)
    w0_d = nc.declare_dram_parameter("w0", [6, 64], F32R, isOutput=False)
    wqk_d = nc.declare_dram_parameter("wqk_dup", [64, 128], BF16, isOutput=False)
    wc1a_d = nc.declare_dram_parameter("wc1a", [128, 128], BF16, isOutput=False)
    wvc_d = nc.declare_dram_parameter("wvc_dup", [128, 128], BF16, isOutput=False)
    wc2_d = nc.declare_dram_parameter("wc2", [128, 128], BF16, isOutput=False)
    wc3_d = nc.declare_dram_parameter("wc3", [128, 2], BF16, isOutput=False)
    sel_d = nc.declare_dram_parameter("sel", [8, 4, 128], BF16, isOutput=False)
    ones2_d = nc.declare_dram_parameter("scoresel", [128, 4, 8], BF16, isOutput=False)
    ones8_d = nc.declare_dram_parameter("ones8", [8, 1], BF16, isOutput=False)
    ones18_d = nc.declare_dram_parameter("ones18", [1, 8], F32R, isOutput=False)
    jsum_d = nc.declare_dram_parameter("jsum", [56, 8], BF16, isOutput=False)
    b0bg_d = nc.declare_dram_parameter("b0bg", [128, 1], F32, isOutput=False)
    bs1_d = nc.declare_dram_parameter("bs1_rep", [128, 1], F32, isOutput=False)
    bs2_d = nc.declare_dram_parameter("bs2_rep", [128, 1], F32, isOutput=False)
    bc1_d = nc.declare_dram_parameter("bc1", [128, 1], F32, isOutput=False)
    bc2_d = nc.declare_dram_parameter("bc2", [128, 1], F32, isOutput=False)
    bc3_d = nc.declare_dram_parameter("bc3", [2, 1], F32, isOutput=False)
    out_d = nc.declare_dram_parameter("out", [2, rpc], F32, isOutput=True)

    with tile.TileContext(nc) as tc:
        consts = tc.alloc_tile_pool(name="consts", bufs=1)
        s1stage_p = tc.alloc_tile_pool(name="s1stage", bufs=2)
        s1T_p = tc.alloc_tile_pool(name="s1T", bufs=2)
        s2_p = tc.alloc_tile_pool(name="s2", bufs=2)
        work_p = tc.alloc_tile_pool(name="work", bufs=3)
        blk_p = tc.alloc_tile_pool(name="blk", bufs=3)
        sm_p = tc.alloc_tile_pool(name="sm", bufs=3)
        psS = tc.alloc_tile_pool(name="psS", bufs=2, space="PSUM")
        psA = tc.alloc_tile_pool(name="psA", bufs=2, space="PSUM")
        psB = tc.alloc_tile_pool(name="psB", bufs=1, space="PSUM")
        psB2 = tc.alloc_tile_pool(name="psB2", bufs=1, space="PSUM")
        psC = tc.alloc_tile_pool(name="psC", bufs=2, space="PSUM")

        # ---- constants / weights to SBUF ----
        wg_sb = consts.tile([P, 8, 64], BF16)
        nc.sync.dma_start(wg_sb, wg_d.rearrange("(c p) m -> p c m", p=P))
        ws1_sb = consts.tile([56, 4, P], BF16)
        nc.sync.dma_start(ws1_sb, ws1_d[:, :, :])
        ws2_sb = consts.tile([P, P], BF16)
        nc.sync.dma_start(ws2_sb, ws2_d[:, :])
        w0_sb = consts.tile([6, 64], F32R)
        nc.sync.dma_start(w0_sb, w0_d[:, :])
        wqk_sb = consts.tile([64, 128], BF16)
        nc.sync.dma_start(wqk_sb, wqk_d[:, :])
        wc1a_sb = consts.tile([P, 128], BF16)
        nc.sync.dma_start(wc1a_sb, wc1a_d[:, :])
        wvc_sb = consts.tile([P, 128], BF16)
        nc.sync.dma_start(wvc_sb, wvc_d[:, :])
        wc2_sb = consts.tile([P, 128], BF16)
        nc.sync.dma_start(wc2_sb, wc2_d[:, :])
        wc3_sb = consts.tile([P, 2], BF16)
        nc.sync.dma_start(wc3_sb, wc3_d[:, :])
        sel_sb = consts.tile([8, 4, P], BF16)
        nc.sync.dma_start(sel_sb, sel_d[:, :, :])
        ones2_sb = consts.tile([P, 4, 8], BF16)
        nc.sync.dma_start(ones2_sb, ones2_d[:, :, :])
        ones8_sb = consts.tile([8, 1], BF16)
        nc.sync.dma_start(ones8_sb, ones8_d[:, :])
        ones18_sb = consts.tile([1, 8], F32R)
        nc.sync.dma_start(ones18_sb, ones18_d[:, :])
        jsum_sb = consts.tile([56, 8], BF16)
        nc.sync.dma_start(jsum_sb, jsum_d[:, :])

        b0bg_sb = consts.tile([P, 1], F32)
        nc.sync.dma_start(b0bg_sb, b0bg_d[:, :])
        bs1_sb = consts.tile([P, 1], F32)
        nc.sync.dma_start(bs1_sb, bs1_d[:, :])
        bs2_sb = consts.tile([P, 1], F32)
        nc.sync.dma_start(bs2_sb, bs2_d[:, :])
        bc1_sb = consts.tile([P, 1], F32)
        nc.sync.dma_start(bc1_sb, bc1_d[:, :])
        bc2_sb = consts.tile([P, 1], F32)
        nc.sync.dma_start(bc2_sb, bc2_d[:, :])
        bc3_sb = consts.tile([2, 1], F32)
        nc.sync.dma_start(bc3_sb, bc3_d[:, :])

        # state0^T loaded once for the whole core (tiny, strided DMA)
        s0T_sb = consts.tile([6, rpc], F32R)
        with nc.allow_non_contiguous_dma(reason="tiny state0 transpose load"):
            nc.sync.dma_start(s0T_sb, s0.rearrange("b f -> f b"))

        ident_f = consts.tile([P, P], F32)
        make_identity(nc, ident_f)
        ident_b = consts.tile([P, P], BF16)
        nc.vector.tensor_copy(ident_b, ident_f)

        out_sb = consts.tile([2, rpc], F32)

        def emit_F1(ms):
            """Loads + state2 pack/transpose + mask rows."""
            row0 = ms * macro
            s1_stage = s1stage_p.tile([P, nb, 1024], BF16, tag="s1stage")
            nc.gpsimd.dma_start(
                s1_stage,
                s1[row0 : row0 + macro, :].rearrange("(o p) f -> p o f", p=P),
            )
            s2_t = s2_p.tile([P, nb, 8, 7], F32, tag="s2")
            nc.sync.dma_start(
                s2_t,
                s2[row0 : row0 + macro, :, :].rearrange("(o p) n j -> p o n j", p=P),
            )
            s2c = sm_p.tile([P, nb, 56], BF16, tag="s2c")
            nc.gpsimd.tensor_copy(s2c, s2_t.rearrange("p o n j -> p o (n j)"))
            s2T_ps = psS.tile([P, nb, P], BF16, tag="psS")
            for o in range(nb):
                nc.tensor.transpose(s2T_ps[0:56, o, :], s2c[:, o, :], ident_b)
            s2T_sb = sm_p.tile([56, nb, P], BF16, tag="s2T")
            nc.vector.tensor_copy(s2T_sb, s2T_ps[0:56])

            nmsum_ps = psC.tile([8, macro], F32, tag="psC")
            nc.tensor.matmul(nmsum_ps, jsum_sb, s2T_sb, start=True, stop=True)
            nm_sb = sm_p.tile([8, macro], BF16, tag="nm")
            nc.vector.tensor_scalar(
                nm_sb, nmsum_ps, 0.0, -1e30, Alu.is_equal, Alu.mult
            )
            return dict(row0=row0, s1_stage=s1_stage, s2T_sb=s2T_sb, nm_sb=nm_sb)

        def emit_F2(st):
            """state1 transposes + env/own + q2."""
            row0 = st["row0"]; s1_stage = st["s1_stage"]
            s1T = s1T_p.tile([P, 8, nb, P], BF16, tag="s1T")
            for o in range(nb):
                t_ps = psS.tile([P, 8, P], BF16, tag="psS")
                for c in range(8):
                    nc.tensor.transpose(
                        t_ps[:, c, :],
                        s1_stage[:, o, c * P : (c + 1) * P],
                        ident_b,
                    )
                dst = s1T[:, :, o, :]
                if o == 1:
                    nc.scalar.copy(dst, t_ps)
                else:
                    nc.vector.tensor_copy(dst, t_ps)

            eo_ps = psB.tile([P, macro], F32, tag="psB")
            nc.tensor.matmul(
                eo_ps[0:64, :], w0_sb, s0T_sb[:, row0 : row0 + macro],
                start=True, stop=True,
            )
            for c in range(8):
                nc.tensor.matmul(
                    eo_ps[64:128, :], wg_sb[:, c], s1T[:, c],
                    start=(c == 0), stop=(c == 7), tile_position=(0, 64),
                )
            concatA = work_p.tile([P, macro], BF16, tag="concatA")
            nc.scalar.activation(concatA, eo_ps, Relu, bias=b0bg_sb)

            q2_ps = psB.tile([P, macro], F32, tag="psB")
            nc.tensor.matmul(q2_ps, wqk_sb, concatA[0:64, :], start=True, stop=True)
            q2_sb = work_p.tile([P, macro], BF16, tag="q2")
            nc.scalar.copy(q2_sb, q2_ps)
            st["concatA"] = concatA
            st["q2_sb"] = q2_sb

        def emit_F3(st):
            """i1, i2, qk."""
            s2T_sb = st["s2T_sb"]; q2_sb = st["q2_sb"]
            i1_sb = blk_p.tile([P, 4, macro], BF16, tag="i1")
            for m in range(4):
                i1_ps = psA.tile([P, macro], F32, tag="psA")
                nc.tensor.matmul(i1_ps, ws1_sb[:, m], s2T_sb, start=True, stop=True)
                dst = i1_sb[:, m, :]
                if m in (0, 3):
                    nc.vector.tensor_scalar(dst, i1_ps, bs1_sb, 0.0, Alu.add, Alu.max)
                else:
                    nc.scalar.activation(dst, i1_ps, Relu, bias=bs1_sb)

            i2_sb = blk_p.tile([P, 4, macro], BF16, tag="i2")
            for m in range(4):
                i2_ps = psA.tile([P, macro], F32, tag="psA")
                nc.tensor.matmul(i2_ps, ws2_sb, i1_sb[:, m, :], start=True, stop=True)
                dst = i2_sb[:, m, :]
                if m in (1, 2):
                    nc.vector.tensor_scalar(dst, i2_ps, bs2_sb, 0.0, Alu.add, Alu.max)
                else:
                    nc.scalar.activation(dst, i2_ps, Relu, bias=bs2_sb)

            qk_sb = blk_p.tile([P, 4, macro], BF16, tag="qk")
            nc.vector.tensor_tensor(
                qk_sb[:, 0:2, :], i2_sb[:, 0:2, :],
                q2_sb[:, None, :].to_broadcast((P, 2, macro)),
                Alu.mult,
            )
            nc.gpsimd.tensor_tensor(
                qk_sb[:, 2:4, :], i2_sb[:, 2:4, :],
                q2_sb[:, None, :].to_broadcast((P, 2, macro)),
                Alu.mult,
            )
            st["i2_sb"] = i2_sb
            st["qk_sb"] = qk_sb

        def emit_B1a(st):
            """Scores -> exp."""
            qk_sb = st["qk_sb"]; nm_sb = st["nm_sb"]
            sc_ps = psC.tile([8, macro], F32, tag="psC")
            for m in range(4):
                nc.tensor.matmul(
                    sc_ps, ones2_sb[:, m, :], qk_sb[:, m, :],
                    start=(m == 0), stop=(m == 3),
                )
            nc.vector.tensor_tensor(sc_ps, sc_ps, nm_sb, Alu.add)
            p8_sb = sm_p.tile([8, macro], BF16, tag="p8")
            nc.scalar.activation(p8_sb, sc_ps, Exp, scale=0.125)
            st["p8_sb"] = p8_sb

        def emit_B1b(st):
            """Softmax denominator -> alpha."""
            p8_sb = st["p8_sb"]
            den_ps = psC.tile([1, macro], F32, tag="psC")
            nc.tensor.matmul(den_ps, ones8_sb, p8_sb, start=True, stop=True)
            rs_sb = sm_p.tile([1, macro], F32R, tag="rs")
            with nc.allow_low_precision(reason="f32r reciprocal, 19-bit ok"):
                nc.vector.reciprocal(rs_sb, den_ps)
            dup_ps = psC.tile([8, macro], F32, tag="psC")
            nc.tensor.matmul(dup_ps, ones18_sb, rs_sb, start=True, stop=True)
            alpha_sb = sm_p.tile([8, macro], BF16, tag="alpha")
            nc.vector.tensor_tensor(alpha_sb, p8_sb, dup_ps, Alu.mult)
            st["alpha_sb"] = alpha_sb

        def emit_B2(st):
            """Weighted i2 + head + tanh."""
            row0 = st["row0"]; concatA = st["concatA"]
            i2_sb = st["i2_sb"]; alpha_sb = st["alpha_sb"]

            cmul_sb = blk_p.tile([P, 4, macro], BF16, tag="cmul")
            abc_sb = sm_p.tile([P, 2, macro], BF16, tag="abc")
            for m in range(4):
                abc_ps = psA.tile([P, macro], F32, tag="psA")
                nc.tensor.matmul(abc_ps, sel_sb[:, m], alpha_sb, start=True, stop=True)
                if m < 2:
                    nc.vector.tensor_tensor(
                        cmul_sb[:, m, :], i2_sb[:, m, :], abc_ps, Alu.mult
                    )
                else:
                    nc.scalar.copy(abc_sb[:, m - 2, :], abc_ps)
            nc.gpsimd.tensor_tensor(
                cmul_sb[:, 2:4, :], i2_sb[:, 2:4, :], abc_sb, Alu.mult
            )

            h1_ps = psB2.tile([P, macro], F32, tag="psB2")
            nc.tensor.matmul(h1_ps, wc1a_sb, concatA, start=True, stop=False)
            for m in range(4):
                nc.tensor.matmul(
                    h1_ps, wvc_sb, cmul_sb[:, m, :],
                    start=False, stop=(m == 3),
                )
            h1_sb = work_p.tile([P, macro], BF16, tag="h1")
            nc.scalar.activation(h1_sb, h1_ps, Relu, bias=bc1_sb)

            h2_ps = psB2.tile([P, macro], F32, tag="psB2")
            nc.tensor.matmul(h2_ps, wc2_sb, h1_sb, start=True, stop=True)
            h2_sb = work_p.tile([P, macro], BF16, tag="h2")
            nc.vector.tensor_scalar(h2_sb, h2_ps, bc2_sb, 0.0, Alu.add, Alu.max)

            o_ps = psB2.tile([2, macro], F32, tag="psB2")
            nc.tensor.matmul(o_ps, wc3_sb, h2_sb, start=True, stop=True)
            nc.scalar.activation(
                out_sb[:, row0 : row0 + macro], o_ps, Tanh, bias=bc3_sb
            )

        # software pipeline, fine-grained: each engine's in-order stream
        # alternates between macro m's latency-critical back half and macro
        # m+1's bulk front half, so dependency stalls are filled.
        prev = None
        for ms in range(nm):
            if prev is not None:
                emit_B1a(prev)
            cur = emit_F1(ms)
            emit_F2(cur)
            if prev is not None:
                emit_B1b(prev)
            emit_F3(cur)
            if prev is not None:
                emit_B2(prev)
            prev = cur
        emit_B1a(prev)
        emit_B1b(prev)
        emit_B2(prev)

        nc.sync.dma_start(out_d[:, :], out_sb)

        for _pool in (psC, psB2, psB, psA, psS, sm_p, blk_p, work_p, s2_p, s1T_p,
                      s1stage_p, consts):
            _pool.release()

    return nc


def prepare_in_maps(inputs):
    bf = ml_dtypes.bfloat16
    f32 = np.float32

    def a(x, dt=f32):
        return np.ascontiguousarray(np.asarray(x), dtype=dt)

    W0 = a(inputs["W0"]); Wg = a(inputs["Wg"])
    Ws1 = a(inputs["Ws1"]); Ws2 = a(inputs["Ws2"])
    Wq = a(inputs["Wq"]); Wk = a(inputs["Wk"]); Wv = a(inputs["Wv"])
    Wc1 = a(inputs["Wc1"]); Wc2 = a(inputs["Wc2"]); Wc3 = a(inputs["Wc3"])

    wqk = Wq @ Wk.T                                   # [64, 64]
    wqk_dup = np.concatenate([wqk, wqk], axis=1)      # [64, 128]
    wvc = Wv @ Wc1[128:192, :]                        # [64, 128]
    wvc_dup = np.concatenate([wvc, wvc], axis=0)      # [128, 128]

    # ws1_blk[7n+j, m, 64r+d] = Ws1[j, d] if n == 2m+r
    ws1_blk = np.zeros((56, 4, 128), dtype=f32)
    sel = np.zeros((8, 4, 128), dtype=f32)
    for n in range(8):
        m, r = n // 2, n % 2
        ws1_blk[7 * n : 7 * n + 7, m, 64 * r : 64 * r + 64] = Ws1
        sel[n, m, 64 * r : 64 * r + 64] = 1.0
    ws2_blk = np.zeros((128, 128), dtype=f32)
    ws2_blk[0:64, 0:64] = Ws2
    ws2_blk[64:128, 64:128] = Ws2

    # scoresel[64r+d, m, n] = 1 iff n == 2m+r (score partition reduce)
    scoresel = np.zeros((128, 4, 8), dtype=f32)
    for n in range(8):
        m, r = n // 2, n % 2
        scoresel[64 * r : 64 * r + 64, m, n] = 1.0
    ones8 = np.ones((8, 1), dtype=f32)
    ones18 = np.ones((1, 8), dtype=f32)
    # jsum[7n+j, n'] = 1 iff n == n'  (per-neighbor feature sums for the mask)
    jsum = np.zeros((56, 8), dtype=f32)
    for n in range(8):
        jsum[7 * n : 7 * n + 7, n] = 1.0

    def col(x):
        return np.ascontiguousarray(np.asarray(x, dtype=f32).reshape(-1, 1))

    b0bg = np.concatenate([col(inputs["b0"]), col(inputs["bg"])], axis=0)
    bs1_rep = np.concatenate([col(inputs["bs1"])] * 2, axis=0)
    bs2_rep = np.concatenate([col(inputs["bs2"])] * 2, axis=0)

    state0 = a(inputs["state0"]); state1 = a(inputs["state1"])
    state2 = a(inputs["state2"])

    shared = {
        "wg_bf": a(Wg, bf),
        "ws1_blk": a(ws1_blk, bf),
        "ws2_blk": a(ws2_blk, bf),
        "w0": W0,
        "wqk_dup": a(wqk_dup, bf),
        "wc1a": a(Wc1[0:128, :], bf),
        "wvc_dup": a(wvc_dup, bf),
        "wc2": a(Wc2, bf),
        "wc3": a(Wc3, bf),
        "sel": a(sel, bf),
        "scoresel": a(scoresel, bf),
        "ones8": a(ones8, bf),
        "ones18": ones18,
        "jsum": a(jsum, bf),
        "b0bg": b0bg,
        "bs1_rep": bs1_rep,
        "bs2_rep": bs2_rep,
        "bc1": col(inputs["bc1"]),
        "bc2": col(inputs["bc2"]),
        "bc3": col(inputs["bc3"]),
    }
    in_maps = []
    for i in range(N_CORES):
        m = dict(shared)
        sl = slice(i * RPC, (i + 1) * RPC)
        m["state0"] = state0[sl]
        m["state1"] = state1[sl]
        m["state2"] = state2[sl]
        in_maps.append(m)
    return in_maps


_NC_CACHE = {}


def get_nc():
    if "nc" not in _NC_CACHE:
        nc = build()
        nc.finalize()
        _NC_CACHE["nc"] = nc
    return _NC_CACHE["nc"]


def kernel(**inputs):
    nc = get_nc()
    in_maps = prepare_in_maps(inputs)
    trace = bool(int(os.environ.get("K_TRACE", "0")))
    try:
        res = run_bass_kernel_spmd(
            nc, in_maps, core_ids=list(range(N_CORES)), trace=trace
        )
    except ModuleNotFoundError:
        res = run_bass_kernel_spmd(nc, in_maps, core_ids=list(range(N_CORES)))
    if res.exec_time_ns is not None:
        print(f"HW exec time: {res.exec_time_ns} ns")
    parts = [np.asarray(res.results[i]["out"], dtype=np.float32).T for i in range(N_CORES)]
    return np.ascontiguousarray(np.concatenate(parts, axis=0))



# revision 11
# speedup vs baseline: 1.1776x; 1.1776x over previous
"""Trainium2 Bass kernel for nn_ActorNetwork (gnn_message_passing).

Pure data-parallel across 8 NeuronCores: each core processes 8192 of the
65536 batch rows; small weights are replicated.

v3 layout: feature-major throughout, neighbor-pair packing as v2, but:
  - state1 is uploaded pre-transposed (bf16 [8, 128, rpc]) so the on-device
    PE transposes + PSUM round-trips for s1 disappear.
  - state0 + state2 are host-packed into one 72-col bf16 block per row
    (56 s2 | 6 s0 | 2 pad | 8 mask-sum slots); a single DVE reduce fills the
    per-neighbor sums, one PE transpose per 128-row block gives the
    feature-major view, and both own_e and i1 read it (own_e via a
    stationary that selects rows 56:62).
  - The attention mask is folded into the score PSUM via a 5th accumulating
    matmul (identity stationary over the -1e30 mask rows).
  - Outputs are PE-transposed into a [128, 128] per-core block so the final
    store is one 512B-per-partition DMA instead of a 2-partition one.
  - Elementwise work is balanced across Act / DVE / Pool with DVE perf modes
    (all-SBUF bf16) where possible.
"""

import os

import numpy as np
import ml_dtypes

import concourse.bass as bass
import concourse.tile as tile
from concourse import bacc
from concourse import mybir
from concourse.bass_utils import run_bass_kernel_spmd
from concourse.masks import make_identity

F32 = mybir.dt.float32
F32R = mybir.dt.float32r
BF16 = mybir.dt.bfloat16

N_CORES = 8
B_FULL = 65536
RPC = B_FULL // N_CORES        # rows per core = 8192
MACRO = 512                    # batch rows per macro tile
P = 128

Relu = mybir.ActivationFunctionType.Relu
Tanh = mybir.ActivationFunctionType.Tanh
Exp = mybir.ActivationFunctionType.Exp
Alu = mybir.AluOpType
AX = mybir.AxisListType


def build(rpc=RPC, macro=MACRO):
    nm = rpc // macro          # macro tiles per core = 16
    nb = macro // P            # 128-row blocks per macro tile = 4

    nc = bacc.Bacc()

    s1t_d = nc.declare_dram_parameter("s1t", [8, P, rpc], BF16, isOutput=False)
    s2a_d = nc.declare_dram_parameter("s2aug", [P, nm, nb * 72], BF16, isOutput=False)
    wg_d = nc.declare_dram_parameter("wg_blk", [P, 8 * 64], BF16, isOutput=False)
    ws1_d = nc.declare_dram_parameter("ws1_blk", [62, 4 * 128], BF16, isOutput=False)
    w0_d = nc.declare_dram_parameter("w0_blk", [62, 64], BF16, isOutput=False)
    ws2_d = nc.declare_dram_parameter("ws2_blk", [128, 128], BF16, isOutput=False)
    wqk_d = nc.declare_dram_parameter("wqk_dup", [64, 128], BF16, isOutput=False)
    wc1a_d = nc.declare_dram_parameter("wc1a", [128, 128], BF16, isOutput=False)
    wvc_d = nc.declare_dram_parameter("wvc_dup", [128, 128], BF16, isOutput=False)
    wc2_d = nc.declare_dram_parameter("wc2", [128, 128], BF16, isOutput=False)
    wc3_d = nc.declare_dram_parameter("wc3", [128, 2], BF16, isOutput=False)
    sel_d = nc.declare_dram_parameter("sel", [8, 4 * 128], BF16, isOutput=False)
    ssel_d = nc.declare_dram_parameter("scoresel", [128, 4 * 8], BF16, isOutput=False)
    ident8_d = nc.declare_dram_parameter("ident8", [8, 8], BF16, isOutput=False)
    ones8_d = nc.declare_dram_parameter("ones8", [8, 1], BF16, isOutput=False)
    ones18_d = nc.declare_dram_parameter("ones18", [1, 8], F32R, isOutput=False)
    b0bg_d = nc.declare_dram_parameter("b0bg", [128, 1], F32, isOutput=False)
    bs1_d = nc.declare_dram_parameter("bs1_rep", [128, 1], F32, isOutput=False)
    bs2_d = nc.declare_dram_parameter("bs2_rep", [128, 1], F32, isOutput=False)
    bc1_d = nc.declare_dram_parameter("bc1", [128, 1], F32, isOutput=False)
    bc2_d = nc.declare_dram_parameter("bc2", [128, 1], F32, isOutput=False)
    bc3_d = nc.declare_dram_parameter("bc3", [2, 1], F32, isOutput=False)
    out_d = nc.declare_dram_parameter("out", [P, nm * nb * 2], F32, isOutput=True)

    with tile.TileContext(nc) as tc:
        consts = tc.alloc_tile_pool(name="consts", bufs=1)
        s1T_p = tc.alloc_tile_pool(name="s1T", bufs=2)
        s2_p = tc.alloc_tile_pool(name="s2", bufs=2)
        sm_p = tc.alloc_tile_pool(name="sm", bufs=3)
        work_p = tc.alloc_tile_pool(name="work", bufs=3)
        blk_p = tc.alloc_tile_pool(name="blk", bufs=3)
        psT = tc.alloc_tile_pool(name="psT", bufs=2, space="PSUM")
        psA = tc.alloc_tile_pool(name="psA", bufs=2, space="PSUM")
        psB = tc.alloc_tile_pool(name="psB", bufs=1, space="PSUM")
        psB2 = tc.alloc_tile_pool(name="psB2", bufs=1, space="PSUM")
        psC = tc.alloc_tile_pool(name="psC", bufs=1, space="PSUM")

        # ---- constants / weights to SBUF ----
        wg_sb = consts.tile([P, 8, 64], BF16)
        nc.sync.dma_start(wg_sb, wg_d.rearrange("p (c m) -> p c m", c=8))
        ws1_sb = consts.tile([62, 4, P], BF16)
        nc.sync.dma_start(ws1_sb, ws1_d.rearrange("p (m k) -> p m k", m=4))
        w0_sb = consts.tile([62, 64], BF16)
        nc.sync.dma_start(w0_sb, w0_d[:, :])
        ws2_sb = consts.tile([P, P], BF16)
        nc.sync.dma_start(ws2_sb, ws2_d[:, :])
        wqk_sb = consts.tile([64, 128], BF16)
        nc.sync.dma_start(wqk_sb, wqk_d[:, :])
        wc1a_sb = consts.tile([P, 128], BF16)
        nc.sync.dma_start(wc1a_sb, wc1a_d[:, :])
        wvc_sb = consts.tile([P, 128], BF16)
        nc.sync.dma_start(wvc_sb, wvc_d[:, :])
        wc2_sb = consts.tile([P, 128], BF16)
        nc.sync.dma_start(wc2_sb, wc2_d[:, :])
        wc3_sb = consts.tile([P, 2], BF16)
        nc.sync.dma_start(wc3_sb, wc3_d[:, :])
        sel_sb = consts.tile([8, 4, P], BF16)
        nc.sync.dma_start(sel_sb, sel_d.rearrange("p (m k) -> p m k", m=4))
        ssel_sb = consts.tile([P, 4, 8], BF16)
        nc.sync.dma_start(ssel_sb, ssel_d.rearrange("p (m k) -> p m k", m=4))
        ident8_sb = consts.tile([8, 8], BF16)
        nc.sync.dma_start(ident8_sb, ident8_d[:, :])
        ones8_sb = consts.tile([8, 1], BF16)
        nc.sync.dma_start(ones8_sb, ones8_d[:, :])
        ones18_sb = consts.tile([1, 8], F32R)
        nc.sync.dma_start(ones18_sb, ones18_d[:, :])

        b0bg_sb = consts.tile([P, 1], F32)
        nc.sync.dma_start(b0bg_sb, b0bg_d[:, :])
        bs1_sb = consts.tile([P, 1], F32)
        nc.sync.dma_start(bs1_sb, bs1_d[:, :])
        bs2_sb = consts.tile([P, 1], F32)
        nc.sync.dma_start(bs2_sb, bs2_d[:, :])
        bc1_sb = consts.tile([P, 1], F32)
        nc.sync.dma_start(bc1_sb, bc1_d[:, :])
        bc2_sb = consts.tile([P, 1], F32)
        nc.sync.dma_start(bc2_sb, bc2_d[:, :])
        bc3_sb = consts.tile([2, 1], F32)
        nc.sync.dma_start(bc3_sb, bc3_d[:, :])

        ident_f = consts.tile([P, P], F32)
        make_identity(nc, ident_f)
        ident_b = consts.tile([P, P], BF16)
        nc.vector.tensor_copy(ident_b, ident_f)

        out128_sb = consts.tile([P, nm, nb, 2], F32)

        def emit_F1(ms):
            """Loads + mask sums + state2 transpose + mask rows."""
            row0 = ms * macro
            s1T = s1T_p.tile([P, 8, macro], BF16, tag="s1T")
            nc.sync.dma_start(
                s1T, s1t_d[:, :, row0 : row0 + macro].rearrange("c p b -> p c b")
            )
            s2a = s2_p.tile([P, nb, 72], BF16, tag="s2a")
            nc.sync.dma_start(
                s2a.rearrange("p o k -> p (o k)"), s2a_d[:, ms, :]
            )
            # per-neighbor feature sums into cols 64:72 (is-padded mask source)
            with nc.allow_low_precision(reason="mask sums only compared to 0"):
                nc.vector.tensor_reduce(
                    s2a[:, :, 64:72],
                    s2a[:, :, 0:56].rearrange("p o (n j) -> p o n j", j=7),
                    AX.X,
                    Alu.add,
                )
            s2T_ps = psT.tile([72, nb, P], BF16, tag="s2T")
            for o in range(nb):
                nc.tensor.transpose(s2T_ps[:, o, :], s2a[:, o, :], ident_b)
            s2T_sb = sm_p.tile([72, nb, P], BF16, tag="s2Tsb")
            nc.scalar.copy(s2T_sb, s2T_ps)
            nm_sb = sm_p.tile([8, macro], BF16, tag="nm")
            nc.vector.tensor_scalar(
                nm_sb,
                s2T_sb[64:72].rearrange("p o k -> p (o k)"),
                0.0,
                -1e30,
                Alu.is_equal,
                Alu.mult,
            )
            return dict(row0=row0, s1T=s1T, s2T_sb=s2T_sb, nm_sb=nm_sb)

        def emit_F2(st):
            """own/env -> concatA -> q2."""
            s1T = st["s1T"]; s2T_sb = st["s2T_sb"]
            eo_ps = psB.tile([P, macro], F32, tag="psB")
            nc.tensor.matmul(
                eo_ps[0:64, :], w0_sb,
                s2T_sb[0:62].rearrange("p o k -> p (o k)"),
                start=True, stop=True,
            )
            for c in range(8):
                nc.tensor.matmul(
                    eo_ps[64:128, :], wg_sb[:, c], s1T[:, c],
                    start=(c == 0), stop=(c == 7), tile_position=(0, 64),
                )
            concatA = work_p.tile([P, macro], BF16, tag="concatA")
            nc.scalar.activation(concatA, eo_ps, Relu, bias=b0bg_sb)

            q2_ps = psB.tile([P, macro], F32, tag="psB")
            nc.tensor.matmul(q2_ps, wqk_sb, concatA[0:64, :], start=True, stop=True)
            q2_sb = work_p.tile([P, macro], BF16, tag="q2")
            nc.scalar.copy(q2_sb, q2_ps)
            st["concatA"] = concatA
            st["q2_sb"] = q2_sb

        def emit_F3(st):
            """i1, i2, qk."""
            s2T_sb = st["s2T_sb"]; q2_sb = st["q2_sb"]
            s2T_flat = s2T_sb[0:62].rearrange("p o k -> p (o k)")
            i1_sb = blk_p.tile([P, 4, macro], BF16, tag="i1")
            for m in range(4):
                i1_ps = psA.tile([P, macro], F32, tag="psA")
                nc.tensor.matmul(i1_ps, ws1_sb[:, m], s2T_flat, start=True, stop=True)
                dst = i1_sb[:, m, :]
                if m in (0, 2):
                    nc.vector.tensor_scalar(dst, i1_ps, bs1_sb, 0.0, Alu.add, Alu.max)
                else:
                    nc.gpsimd.tensor_scalar(dst, i1_ps, bs1_sb, 0.0, Alu.add, Alu.max)

            i2_sb = blk_p.tile([P, 4, macro], BF16, tag="i2")
            for m in range(4):
                i2_ps = psA.tile([P, macro], F32, tag="psA")
                nc.tensor.matmul(i2_ps, ws2_sb, i1_sb[:, m, :], start=True, stop=True)
                dst = i2_sb[:, m, :]
                if m in (0, 2):
                    nc.scalar.activation(dst, i2_ps, Relu, bias=bs2_sb)
                else:
                    nc.gpsimd.tensor_scalar(dst, i2_ps, bs2_sb, 0.0, Alu.add, Alu.max)

            qk_sb = blk_p.tile([P, 4, macro], BF16, tag="qk")
            nc.vector.tensor_tensor(
                qk_sb, i2_sb,
                q2_sb[:, None, :].to_broadcast((P, 4, macro)),
                Alu.mult,
            )
            st["i2_sb"] = i2_sb
            st["qk_sb"] = qk_sb

        def emit_B1a(st):
            """Scores (incl. mask) -> exp."""
            qk_sb = st["qk_sb"]; nm_sb = st["nm_sb"]
            sc_ps = psC.tile([8, macro], F32, tag="psC")
            for m in range(4):
                nc.tensor.matmul(
                    sc_ps, ssel_sb[:, m, :], qk_sb[:, m, :],
                    start=(m == 0), stop=False,
                )
            nc.tensor.matmul(sc_ps, ident8_sb, nm_sb, start=False, stop=True)
            p8_sb = sm_p.tile([8, macro], BF16, tag="p8")
            nc.scalar.activation(p8_sb, sc_ps, Exp, scale=0.125)
            st["p8_sb"] = p8_sb

        def emit_B1b(st):
            """Softmax denominator -> alpha."""
            p8_sb = st["p8_sb"]
            den_ps = psC.tile([1, macro], F32, tag="psC")
            nc.tensor.matmul(den_ps, ones8_sb, p8_sb, start=True, stop=True)
            rs_sb = sm_p.tile([1, macro], F32R, tag="rs")
            with nc.allow_low_precision(reason="f32r reciprocal, 19-bit ok"):
                nc.vector.reciprocal(rs_sb, den_ps)
            dup_ps = psC.tile([8, macro], F32, tag="psC")
            nc.tensor.matmul(dup_ps, ones18_sb, rs_sb, start=True, stop=True)
            alpha_sb = sm_p.tile([8, macro], BF16, tag="alpha")
            nc.vector.tensor_tensor(alpha_sb, p8_sb, dup_ps, Alu.mult)
            st["alpha_sb"] = alpha_sb

        def emit_B2(st):
            """Weighted i2 + head + tanh + output transpose."""
            ms = st["row0"] // macro
            concatA = st["concatA"]
            i2_sb = st["i2_sb"]; alpha_sb = st["alpha_sb"]

            cmul_sb = blk_p.tile([P, 4, macro], BF16, tag="cmul")
            for m in range(4):
                abc_ps = psA.tile([P, macro], F32, tag="psA")
                nc.tensor.matmul(abc_ps, sel_sb[:, m], alpha_sb, start=True, stop=True)
                if m in (0, 2):
                    nc.vector.tensor_tensor(
                        cmul_sb[:, m, :], i2_sb[:, m, :], abc_ps, Alu.mult
                    )
                else:
                    nc.gpsimd.tensor_tensor(
                        cmul_sb[:, m, :], i2_sb[:, m, :], abc_ps, Alu.mult
                    )

            h1_ps = psB2.tile([P, macro], F32, tag="psB2")
            nc.tensor.matmul(h1_ps, wc1a_sb, concatA, start=True, stop=False)
            for m in range(4):
                nc.tensor.matmul(
                    h1_ps, wvc_sb, cmul_sb[:, m, :],
                    start=False, stop=(m == 3),
                )
            h1_sb = work_p.tile([P, macro], BF16, tag="h1")
            nc.scalar.activation(h1_sb, h1_ps, Relu, bias=bc1_sb)

            h2_ps = psB2.tile([P, macro], F32, tag="psB2")
            nc.tensor.matmul(h2_ps, wc2_sb, h1_sb, start=True, stop=True)
            h2_sb = work_p.tile([P, macro], BF16, tag="h2")
            nc.vector.tensor_scalar(h2_sb, h2_ps, bc2_sb, 0.0, Alu.add, Alu.max)

            o_ps = psC.tile([2, macro], F32, tag="psC")
            nc.tensor.matmul(o_ps, wc3_sb, h2_sb, start=True, stop=True)
            o_sb = sm_p.tile([2, macro], F32, tag="osb")
            nc.scalar.activation(o_sb, o_ps, Tanh, bias=bc3_sb)
            oT_ps = psB2.tile([P, nb, 2], F32, tag="oT")
            for o in range(nb):
                nc.tensor.transpose(
                    oT_ps[:, o, :], o_sb[:, o * P : (o + 1) * P],
                    ident_f[0:2, 0:2],
                )
            nc.vector.tensor_copy(out128_sb[:, ms], oT_ps)

        # software pipeline: each engine's in-order stream alternates between
        # macro m's latency-critical back half and macro m+1's bulk front half.
        prev = None
        for ms in range(nm):
            if prev is not None:
                emit_B1a(prev)
            cur = emit_F1(ms)
            emit_F2(cur)
            if prev is not None:
                emit_B1b(prev)
            emit_F3(cur)
            if prev is not None:
                emit_B2(prev)
            prev = cur
        emit_B1a(prev)
        emit_B1b(prev)
        emit_B2(prev)

        nc.sync.dma_start(
            out_d[:, :], out128_sb.rearrange("p a o k -> p (a o k)")
        )

        for _pool in (psC, psB2, psB, psA, psT, blk_p, work_p, sm_p, s2_p,
                      s1T_p, consts):
            _pool.release()

    return nc


def prepare_in_maps(inputs):
    bf = ml_dtypes.bfloat16
    f32 = np.float32
    nm = RPC // MACRO
    nb = MACRO // P

    def a(x, dt=f32):
        return np.ascontiguousarray(np.asarray(x), dtype=dt)

    W0 = a(inputs["W0"]); Wg = a(inputs["Wg"])
    Ws1 = a(inputs["Ws1"]); Ws2 = a(inputs["Ws2"])
    Wq = a(inputs["Wq"]); Wk = a(inputs["Wk"]); Wv = a(inputs["Wv"])
    Wc1 = a(inputs["Wc1"]); Wc2 = a(inputs["Wc2"]); Wc3 = a(inputs["Wc3"])

    wqk = Wq @ Wk.T                                   # [64, 64]
    wqk_dup = np.concatenate([wqk, wqk], axis=1)      # [64, 128]
    wvc = Wv @ Wc1[128:192, :]                        # [64, 128]
    wvc_dup = np.concatenate([wvc, wvc], axis=0)      # [128, 128]

    # ws1_blk[7n+j, m, 64r+d] = Ws1[j, d] if n == 2m+r; rows 56:62 zero
    ws1_blk = np.zeros((62, 4, 128), dtype=f32)
    sel = np.zeros((8, 4, 128), dtype=f32)
    for n in range(8):
        m, r = n // 2, n % 2
        ws1_blk[7 * n : 7 * n + 7, m, 64 * r : 64 * r + 64] = Ws1
        sel[n, m, 64 * r : 64 * r + 64] = 1.0
    w0_blk = np.zeros((62, 64), dtype=f32)
    w0_blk[56:62, :] = W0
    ws2_blk = np.zeros((128, 128), dtype=f32)
    ws2_blk[0:64, 0:64] = Ws2
    ws2_blk[64:128, 64:128] = Ws2

    # scoresel[64r+d, m, n] = 1 iff n == 2m+r (score partition reduce)
    scoresel = np.zeros((128, 4, 8), dtype=f32)
    for n in range(8):
        m, r = n // 2, n % 2
        scoresel[64 * r : 64 * r + 64, m, n] = 1.0
    ones8 = np.ones((8, 1), dtype=f32)
    ones18 = np.ones((1, 8), dtype=f32)

    def col(x):
        return np.ascontiguousarray(np.asarray(x, dtype=f32).reshape(-1, 1))

    b0bg = np.concatenate([col(inputs["b0"]), col(inputs["bg"])], axis=0)
    bs1_rep = np.concatenate([col(inputs["bs1"])] * 2, axis=0)
    bs2_rep = np.concatenate([col(inputs["bs2"])] * 2, axis=0)

    state0 = a(inputs["state0"]); state1 = a(inputs["state1"])
    state2 = a(inputs["state2"])

    # state1 pre-transposed: s1t[c, p, b] = state1[b, 128c + p], bf16
    s1t_full = np.ascontiguousarray(
        state1.T.reshape(8, P, B_FULL).astype(bf)
    )

    # s2aug: per row 72 cols = 56 s2 | 6 s0 | 2 pad | 8 sum slots (zeros)
    s2aug = np.zeros((B_FULL, 72), dtype=bf)
    s2aug[:, 0:56] = state2.reshape(B_FULL, 56).astype(bf)
    s2aug[:, 56:62] = state0.astype(bf)
    # [core, ms, o, p, 72] -> [core, p, ms, (o 72)]
    s2aug_blk = np.ascontiguousarray(
        s2aug.reshape(N_CORES, nm, nb, P, 72).transpose(0, 3, 1, 2, 4)
        .reshape(N_CORES, P, nm, nb * 72)
    )

    shared = {
        "wg_blk": a(Wg, bf).reshape(8, P, 64).transpose(1, 0, 2).reshape(P, 8 * 64),
        "ws1_blk": a(ws1_blk, bf).reshape(62, 4 * 128),
        "w0_blk": a(w0_blk, bf),
        "ws2_blk": a(ws2_blk, bf),
        "wqk_dup": a(wqk_dup, bf),
        "wc1a": a(Wc1[0:128, :], bf),
        "wvc_dup": a(wvc_dup, bf),
        "wc2": a(Wc2, bf),
        "wc3": a(Wc3, bf),
        "sel": a(sel, bf).reshape(8, 4 * 128),
        "scoresel": a(scoresel, bf).reshape(128, 4 * 8),
        "ident8": a(np.eye(8, dtype=f32), bf),
        "ones8": a(ones8, bf),
        "ones18": ones18,
        "b0bg": b0bg,
        "bs1_rep": bs1_rep,
        "bs2_rep": bs2_rep,
        "bc1": col(inputs["bc1"]),
        "bc2": col(inputs["bc2"]),
        "bc3": col(inputs["bc3"]),
    }
    in_maps = []
    for i in range(N_CORES):
        m = dict(shared)
        m["s1t"] = np.ascontiguousarray(s1t_full[:, :, i * RPC : (i + 1) * RPC])
        m["s2aug"] = s2aug_blk[i]
        in_maps.append(m)
    return in_maps


def unshard_out(res_core):
    """[128, nm*nb*2] f32 -> [rpc, 2]; out128[p, ms, o, a] = row ms*512+o*128+p."""
    nm = RPC // MACRO
    nb = MACRO // P
    arr = np.asarray(res_core, dtype=np.float32).reshape(P, nm, nb, 2)
    return np.ascontiguousarray(
        arr.transpose(1, 2, 0, 3).reshape(RPC, 2)
    )


_NC_CACHE = {}


def get_nc():
    if "nc" not in _NC_CACHE:
        nc = build()
        nc.finalize()
        _NC_CACHE["nc"] = nc
    return _NC_CACHE["nc"]


def kernel(**inputs):
    nc = get_nc()
    in_maps = prepare_in_maps(inputs)
    trace = bool(int(os.environ.get("K_TRACE", "0")))
    try:
        res = run_bass_kernel_spmd(
            nc, in_maps, core_ids=list(range(N_CORES)), trace=trace
        )
    except ModuleNotFoundError:
        res = run_bass_kernel_spmd(nc, in_maps, core_ids=list(range(N_CORES)))
    if res.exec_time_ns is not None:
        print(f"HW exec time: {res.exec_time_ns} ns")
    parts = [unshard_out(res.results[i]["out"]) for i in range(N_CORES)]
    return np.ascontiguousarray(np.concatenate(parts, axis=0))


# revision 15
# speedup vs baseline: 1.1905x; 1.0109x over previous
"""Trainium2 Bass kernel for nn_ActorNetwork (gnn_message_passing).

Pure data-parallel across 8 NeuronCores: each core processes 8192 of the
65536 batch rows; small weights are replicated.

v4 layout: feature-major, neighbor-pair packing; vs v2 baseline:
  - state1 uploaded pre-transposed (bf16 [8, 128, rpc]) - no on-device s1
    transposes.
  - state0 + state2 host-packed into one 72-col bf16 block per row
    (56 s2 | 6 s0 | 2 pad | 8 sum slots); a DVE reduce fills per-neighbor
    sums, one PE transpose per 128-row block gives the feature-major view;
    own_e reads rows 56:62 via its own stationary.
  - attention mask folded into the score PSUM via a 5th accumulating matmul.
  - outputs PE-transposed into a [128, 128] per-core block -> one 512B/part
    store DMA.
  - all small weights packed into two DMAs issued on the Act queue so the
    SP queue starts streaming state immediately.
  - emission order tuned so PE never waits on the softmax scalar chain:
    env runs before den/dup, the s2 transpose block after, and the output
    transposes are deferred one phase.
"""

import os

import numpy as np
import ml_dtypes

import concourse.bass as bass
import concourse.tile as tile
from concourse import bacc
from concourse import mybir
from concourse.bass_utils import run_bass_kernel_spmd

F32 = mybir.dt.float32
F32R = mybir.dt.float32r
BF16 = mybir.dt.bfloat16

N_CORES = 8
B_FULL = 65536
RPC = B_FULL // N_CORES        # rows per core = 8192
MACRO = 512                    # batch rows per macro tile
P = 128

Relu = mybir.ActivationFunctionType.Relu
Tanh = mybir.ActivationFunctionType.Tanh
Exp = mybir.ActivationFunctionType.Exp
Alu = mybir.AluOpType
AX = mybir.AxisListType

# column layout of the packed bf16 weight block [128, WPK_COLS]
_off = 0
def _span(n):
    global _off
    s = (_off, _off + n)
    _off += n
    return s
WG_S = _span(512)        # [128, (8c, 64)]
WS1_S = _span(512)       # [62, (4m, 128)]
W0_S = _span(64)         # [62, 64]
WS2_S = _span(128)       # [128, 128]
WQK_S = _span(128)       # [64, 128]
WC1A_S = _span(128)      # [128, 128]
WVC_S = _span(128)       # [128, 128]
WC2_S = _span(128)       # [128, 128]
WC3_S = _span(2)         # [128, 2]
SEL_S = _span(512)       # [8, (4m, 128)]
SSEL_S = _span(32)       # [128, (4m, 8)]
ID8_S = _span(8)         # [8, 8]
ONES8_S = _span(1)       # [8, 1]
IDB_S = _span(128)       # [128, 128] bf16 identity
WPK_COLS = _off


def build(rpc=RPC, macro=MACRO):
    nm = rpc // macro          # macro tiles per core = 16
    nb = macro // P            # 128-row blocks per macro tile = 4

    nc = bacc.Bacc()

    s1t_d = nc.declare_dram_parameter("s1t", [8, P, rpc], BF16, isOutput=False)
    s2a_d = nc.declare_dram_parameter("s2aug", [P, nm, nb * 72], BF16, isOutput=False)
    wpk_d = nc.declare_dram_parameter("wpk", [P, WPK_COLS], BF16, isOutput=False)
    bpk_d = nc.declare_dram_parameter("bpk", [P, 8], F32, isOutput=False)
    ones18_d = nc.declare_dram_parameter("ones18", [1, 8], F32R, isOutput=False)
    out_d = nc.declare_dram_parameter("out", [P, nm * nb * 2], F32, isOutput=True)

    with tile.TileContext(nc) as tc:
        consts = tc.alloc_tile_pool(name="consts", bufs=1)
        s1T_p = tc.alloc_tile_pool(name="s1T", bufs=2)
        s2_p = tc.alloc_tile_pool(name="s2", bufs=2)
        sm_p = tc.alloc_tile_pool(name="sm", bufs=3)
        work_p = tc.alloc_tile_pool(name="work", bufs=3)
        blk_p = tc.alloc_tile_pool(name="blk", bufs=3)
        psT = tc.alloc_tile_pool(name="psT", bufs=1, space="PSUM")
        psA = tc.alloc_tile_pool(name="psA", bufs=3, space="PSUM")
        psB = tc.alloc_tile_pool(name="psB", bufs=1, space="PSUM")
        psB2 = tc.alloc_tile_pool(name="psB2", bufs=1, space="PSUM")
        psC = tc.alloc_tile_pool(name="psC", bufs=1, space="PSUM")

        # ---- packed constants (Act queue; SP starts on state immediately) --
        wpk = consts.tile([P, WPK_COLS], BF16)
        nc.scalar.dma_start(wpk, wpk_d[:, :])
        bpk = consts.tile([P, 8], F32)
        nc.scalar.dma_start(bpk, bpk_d[:, :])
        ones18_sb = consts.tile([1, 8], F32R)
        nc.scalar.dma_start(ones18_sb, ones18_d[:, :])

        def W(span, rows=P):
            return wpk[0:rows, span[0] : span[1]]

        wg_sb = W(WG_S).rearrange("p (c m) -> p c m", c=8)
        ws1_sb = W(WS1_S, 62).rearrange("p (m k) -> p m k", m=4)
        w0_sb = W(W0_S, 62)
        ws2_sb = W(WS2_S)
        wqk_sb = W(WQK_S, 64)
        wc1a_sb = W(WC1A_S)
        wvc_sb = W(WVC_S)
        wc2_sb = W(WC2_S)
        wc3_sb = W(WC3_S)
        sel_sb = W(SEL_S, 8).rearrange("p (m k) -> p m k", m=4)
        ssel_sb = W(SSEL_S).rearrange("p (m k) -> p m k", m=4)
        ident8_sb = W(ID8_S, 8)
        ones8_sb = W(ONES8_S, 8)
        ident_b = W(IDB_S)

        b0bg_sb = bpk[:, 0:1]
        bs1_sb = bpk[:, 1:2]
        bs2_sb = bpk[:, 2:3]
        bc1_sb = bpk[:, 3:4]
        bc2_sb = bpk[:, 4:5]
        bc3_sb = bpk[0:2, 5:6]
        ident2_f = bpk[0:2, 6:8]

        out128_sb = consts.tile([P, nm, nb, 2], F32)

        def emit_F1(ms):
            """State loads for macro ms."""
            row0 = ms * macro
            s1T = s1T_p.tile([P, 8, macro], BF16, tag="s1T")
            nc.sync.dma_start(
                s1T, s1t_d[:, :, row0 : row0 + macro].rearrange("c p b -> p c b")
            )
            s2a = s2_p.tile([P, nb, 72], BF16, tag="s2a")
            nc.sync.dma_start(
                s2a.rearrange("p o k -> p (o k)"), s2a_d[:, ms, :]
            )
            return dict(row0=row0, s1T=s1T, s2a=s2a)

        def emit_F2a(st):
            """env matmuls (only need s1T)."""
            eo_ps = psB.tile([P, macro], F32, tag="psB")
            for c in range(8):
                nc.tensor.matmul(
                    eo_ps[64:128, :], wg_sb[:, c], st["s1T"][:, c],
                    start=(c == 0), stop=(c == 7), tile_position=(0, 64),
                )
            st["eo_ps"] = eo_ps

        def emit_F1b(st):
            """Mask sums + state2 transpose + mask rows."""
            s2a = st["s2a"]
            with nc.allow_low_precision(reason="mask sums only compared to 0"):
                nc.vector.tensor_reduce(
                    s2a[:, :, 64:72],
                    s2a[:, :, 0:56].rearrange("p o (n j) -> p o n j", j=7),
                    AX.X,
                    Alu.add,
                )
            s2T_ps = psT.tile([72, nb, P], BF16, tag="s2T")
            for o in range(nb):
                nc.tensor.transpose(s2T_ps[:, o, :], s2a[:, o, :], ident_b)
            s2T_sb = sm_p.tile([72, nb, P], BF16, tag="s2Tsb")
            nc.scalar.copy(s2T_sb, s2T_ps)
            nm_sb = sm_p.tile([8, macro], BF16, tag="nm")
            nc.gpsimd.tensor_scalar(
                nm_sb,
                s2T_sb[64:72].rearrange("p o k -> p (o k)"),
                0.0,
                -1e30,
                Alu.is_equal,
                Alu.mult,
            )
            st["s2T_sb"] = s2T_sb
            st["nm_sb"] = nm_sb

        def emit_F2b(st):
            """own matmul -> concatA -> q2."""
            eo_ps = st["eo_ps"]
            nc.tensor.matmul(
                eo_ps[0:64, :], w0_sb,
                st["s2T_sb"][0:62].rearrange("p o k -> p (o k)"),
                start=True, stop=True,
            )
            concatA = work_p.tile([P, macro], BF16, tag="concatA")
            nc.scalar.activation(concatA, eo_ps, Relu, bias=b0bg_sb)

            q2_ps = psB.tile([P, macro], F32, tag="psB")
            nc.tensor.matmul(q2_ps, wqk_sb, concatA[0:64, :], start=True, stop=True)
            q2_sb = work_p.tile([P, macro], BF16, tag="q2")
            nc.scalar.copy(q2_sb, q2_ps)
            st["concatA"] = concatA
            st["q2_sb"] = q2_sb

        def emit_F3a(st):
            """i1."""
            s2T_flat = st["s2T_sb"][0:62].rearrange("p o k -> p (o k)")
            i1_sb = blk_p.tile([P, 4, macro], BF16, tag="i1")
            for m in range(4):
                i1_ps = psA.tile([P, macro], F32, tag="psA")
                nc.tensor.matmul(i1_ps, ws1_sb[:, m], s2T_flat, start=True, stop=True)
                dst = i1_sb[:, m, :]
                if m == 0:
                    nc.scalar.activation(dst, i1_ps, Relu, bias=bs1_sb)
                elif m == 1:
                    nc.vector.tensor_scalar(dst, i1_ps, bs1_sb, 0.0, Alu.add, Alu.max)
                else:
                    nc.gpsimd.tensor_scalar(dst, i1_ps, bs1_sb, 0.0, Alu.add, Alu.max)
            st["i1_sb"] = i1_sb

        def emit_F3b(st):
            """i2, qk."""
            i1_sb = st["i1_sb"]; q2_sb = st["q2_sb"]
            i2_sb = blk_p.tile([P, 4, macro], BF16, tag="i2")
            for m in range(4):
                i2_ps = psA.tile([P, macro], F32, tag="psA")
                nc.tensor.matmul(i2_ps, ws2_sb, i1_sb[:, m, :], start=True, stop=True)
                dst = i2_sb[:, m, :]
                if m in (0, 1):
                    nc.scalar.activation(dst, i2_ps, Relu, bias=bs2_sb)
                else:
                    nc.gpsimd.tensor_scalar(dst, i2_ps, bs2_sb, 0.0, Alu.add, Alu.max)

            qk_sb = blk_p.tile([P, 4, macro], BF16, tag="qk")
            nc.vector.tensor_tensor(
                qk_sb, i2_sb,
                q2_sb[:, None, :].to_broadcast((P, 4, macro)),
                Alu.mult,
            )
            st["i2_sb"] = i2_sb
            st["qk_sb"] = qk_sb

        def emit_B1a(st):
            """Scores (incl. mask) -> exp."""
            sc_ps = psC.tile([8, macro], F32, tag="psC")
            for m in range(4):
                nc.tensor.matmul(
                    sc_ps, ssel_sb[:, m, :], st["qk_sb"][:, m, :],
                    start=(m == 0), stop=False,
                )
            nc.tensor.matmul(sc_ps, ident8_sb, st["nm_sb"], start=False, stop=True)
            p8_sb = sm_p.tile([8, macro], BF16, tag="p8")
            nc.scalar.activation(p8_sb, sc_ps, Exp, scale=0.125)
            st["p8_sb"] = p8_sb

        def emit_B1b(st):
            """Softmax denominator -> alpha."""
            den_ps = psC.tile([1, macro], F32, tag="psC")
            nc.tensor.matmul(den_ps, ones8_sb, st["p8_sb"], start=True, stop=True)
            rs_sb = sm_p.tile([1, macro], F32R, tag="rs")
            with nc.allow_low_precision(reason="f32r reciprocal, 19-bit ok"):
                nc.vector.reciprocal(rs_sb, den_ps)
            dup_ps = psC.tile([8, macro], F32, tag="psC")
            nc.tensor.matmul(dup_ps, ones18_sb, rs_sb, start=True, stop=True)
            alpha_sb = sm_p.tile([8, macro], BF16, tag="alpha")
            nc.vector.tensor_tensor(alpha_sb, st["p8_sb"], dup_ps, Alu.mult)
            st["alpha_sb"] = alpha_sb

        def emit_B2a(st):
            """Weighted i2 -> h1."""
            i2_sb = st["i2_sb"]; alpha_sb = st["alpha_sb"]

            cmul_sb = blk_p.tile([P, 4, macro], BF16, tag="cmul")
            abc = []
            for m in range(4):
                abc_ps = psA.tile([P, macro], F32, tag="psA")
                nc.tensor.matmul(abc_ps, sel_sb[:, m], alpha_sb, start=True, stop=True)
                abc.append(abc_ps)
            for m in range(4):
                if m == 3:
                    nc.gpsimd.tensor_tensor(
                        cmul_sb[:, m, :], i2_sb[:, m, :], abc[m], Alu.mult
                    )
                else:
                    nc.vector.tensor_tensor(
                        cmul_sb[:, m, :], i2_sb[:, m, :], abc[m], Alu.mult
                    )

            h1_ps = psB2.tile([P, macro], F32, tag="psB2")
            nc.tensor.matmul(h1_ps, wc1a_sb, st["concatA"], start=True, stop=False)
            for m in range(4):
                nc.tensor.matmul(
                    h1_ps, wvc_sb, cmul_sb[:, m, :],
                    start=False, stop=(m == 3),
                )
            h1_sb = work_p.tile([P, macro], BF16, tag="h1")
            nc.scalar.activation(h1_sb, h1_ps, Relu, bias=bc1_sb)
            st["h1_sb"] = h1_sb

        def emit_B2b(st):
            """h2."""
            h2_ps = psB2.tile([P, macro], F32, tag="psB2")
            nc.tensor.matmul(h2_ps, wc2_sb, st["h1_sb"], start=True, stop=True)
            h2_sb = work_p.tile([P, macro], BF16, tag="h2")
            nc.vector.tensor_scalar(h2_sb, h2_ps, bc2_sb, 0.0, Alu.add, Alu.max)
            st["h2_sb"] = h2_sb

        def emit_B2c(st):
            """Output head + tanh."""
            o_ps = psC.tile([2, macro], F32, tag="psC")
            nc.tensor.matmul(o_ps, wc3_sb, st["h2_sb"], start=True, stop=True)
            o_sb = sm_p.tile([2, macro], F32, tag="osb")
            nc.scalar.activation(o_sb, o_ps, Tanh, bias=bc3_sb)
            st["o_sb"] = o_sb

        def emit_B3(st):
            """Output transpose into the 128-partition store block (deferred
            one phase so PE never waits on tanh)."""
            ms = st["row0"] // macro
            oT_ps = psB2.tile([P, nb, 2], F32, tag="oT")
            for o in range(nb):
                nc.tensor.transpose(
                    oT_ps[:, o, :], st["o_sb"][:, o * P : (o + 1) * P], ident2_f
                )
            nc.vector.tensor_copy(out128_sb[:, ms], oT_ps)

        # software pipeline across macros; state DMAs prefetched one macro
        # ahead so env never waits on the s1T load.
        nxt = emit_F1(0)
        prev = None
        done = None
        for ms in range(nm):
            cur = nxt
            if prev is not None:
                emit_B1a(prev)
            if done is not None:
                emit_B3(done)
            if ms + 1 < nm:
                nxt = emit_F1(ms + 1)
            emit_F2a(cur)
            if prev is not None:
                emit_B1b(prev)
            emit_F1b(cur)
            if prev is not None:
                emit_B2a(prev)
            emit_F2b(cur)
            if prev is not None:
                emit_B2b(prev)
            emit_F3a(cur)
            if prev is not None:
                emit_B2c(prev)
            emit_F3b(cur)
            done = prev
            prev = cur
        emit_B1a(prev)
        emit_B3(done)
        emit_B1b(prev)
        emit_B2a(prev)
        emit_B2b(prev)
        emit_B2c(prev)
        emit_B3(prev)

        nc.sync.dma_start(
            out_d[:, :], out128_sb.rearrange("p a o k -> p (a o k)")
        )

        for _pool in (psC, psB2, psB, psA, psT, blk_p, work_p, sm_p, s2_p,
                      s1T_p, consts):
            _pool.release()

    return nc


def prepare_in_maps(inputs):
    bf = ml_dtypes.bfloat16
    f32 = np.float32
    nm = RPC // MACRO
    nb = MACRO // P

    def a(x, dt=f32):
        return np.ascontiguousarray(np.asarray(x), dtype=dt)

    W0 = a(inputs["W0"]); Wg = a(inputs["Wg"])
    Ws1 = a(inputs["Ws1"]); Ws2 = a(inputs["Ws2"])
    Wq = a(inputs["Wq"]); Wk = a(inputs["Wk"]); Wv = a(inputs["Wv"])
    Wc1 = a(inputs["Wc1"]); Wc2 = a(inputs["Wc2"]); Wc3 = a(inputs["Wc3"])

    wqk = Wq @ Wk.T                                   # [64, 64]
    wvc = Wv @ Wc1[128:192, :]                        # [64, 128]

    wpk = np.zeros((P, WPK_COLS), dtype=f32)

    def put(span, arr, rows=None):
        arr = np.asarray(arr, dtype=f32)
        r = arr.shape[0] if rows is None else rows
        wpk[0:r, span[0] : span[0] + arr.shape[1]] = arr

    # wg: [128, (8c, 64)] with wg[p, c, :] = Wg[128c + p, :]
    put(WG_S, Wg.reshape(8, P, 64).transpose(1, 0, 2).reshape(P, 512))
    # ws1_blk[7n+j, m, 64r+d] = Ws1[j, d] if n == 2m+r (rows 56:62 zero)
    ws1_blk = np.zeros((62, 4, 128), dtype=f32)
    sel = np.zeros((8, 4, 128), dtype=f32)
    for n in range(8):
        m, r = n // 2, n % 2
        ws1_blk[7 * n : 7 * n + 7, m, 64 * r : 64 * r + 64] = Ws1
        sel[n, m, 64 * r : 64 * r + 64] = 1.0
    put(WS1_S, ws1_blk.reshape(62, 512))
    w0_blk = np.zeros((62, 64), dtype=f32)
    w0_blk[56:62, :] = W0
    put(W0_S, w0_blk)
    ws2_blk = np.zeros((128, 128), dtype=f32)
    ws2_blk[0:64, 0:64] = Ws2
    ws2_blk[64:128, 64:128] = Ws2
    put(WS2_S, ws2_blk)
    put(WQK_S, np.concatenate([wqk, wqk], axis=1))
    put(WC1A_S, Wc1[0:128, :])
    put(WVC_S, np.concatenate([wvc, wvc], axis=0))
    put(WC2_S, Wc2)
    put(WC3_S, Wc3)
    put(SEL_S, sel.reshape(8, 512))
    scoresel = np.zeros((128, 4, 8), dtype=f32)
    for n in range(8):
        m, r = n // 2, n % 2
        scoresel[64 * r : 64 * r + 64, m, n] = 1.0
    put(SSEL_S, scoresel.reshape(128, 32))
    put(ID8_S, np.eye(8, dtype=f32))
    put(ONES8_S, np.ones((8, 1), dtype=f32))
    put(IDB_S, np.eye(128, dtype=f32))

    def col(x):
        return np.ascontiguousarray(np.asarray(x, dtype=f32).reshape(-1, 1))

    bpk = np.zeros((P, 8), dtype=f32)
    bpk[:, 0:1] = np.concatenate([col(inputs["b0"]), col(inputs["bg"])], axis=0)
    bpk[:, 1:2] = np.concatenate([col(inputs["bs1"])] * 2, axis=0)
    bpk[:, 2:3] = np.concatenate([col(inputs["bs2"])] * 2, axis=0)
    bpk[:, 3:4] = col(inputs["bc1"])
    bpk[:, 4:5] = col(inputs["bc2"])
    bpk[0:2, 5:6] = col(inputs["bc3"])
    bpk[0:2, 6:8] = np.eye(2, dtype=f32)

    state0 = a(inputs["state0"]); state1 = a(inputs["state1"])
    state2 = a(inputs["state2"])

    # state1 pre-transposed: s1t[c, p, b] = state1[b, 128c + p], bf16
    s1t_full = np.ascontiguousarray(
        state1.T.reshape(8, P, B_FULL).astype(bf)
    )

    # s2aug: per row 72 cols = 56 s2 | 6 s0 | 2 pad | 8 sum slots (zeros)
    s2aug = np.zeros((B_FULL, 72), dtype=bf)
    s2aug[:, 0:56] = state2.reshape(B_FULL, 56).astype(bf)
    s2aug[:, 56:62] = state0.astype(bf)
    # [core, ms, o, p, 72] -> [core, p, ms, (o 72)]
    s2aug_blk = np.ascontiguousarray(
        s2aug.reshape(N_CORES, nm, nb, P, 72).transpose(0, 3, 1, 2, 4)
        .reshape(N_CORES, P, nm, nb * 72)
    )

    shared = {
        "wpk": a(wpk, bf),
        "bpk": bpk,
        "ones18": np.ones((1, 8), dtype=f32),
    }
    in_maps = []
    for i in range(N_CORES):
        m = dict(shared)
        m["s1t"] = np.ascontiguousarray(s1t_full[:, :, i * RPC : (i + 1) * RPC])
        m["s2aug"] = s2aug_blk[i]
        in_maps.append(m)
    return in_maps


def unshard_out(res_core):
    """[128, nm*nb*2] f32 -> [rpc, 2]; out128[p, ms, o, a] = row ms*512+o*128+p."""
    nm = RPC // MACRO
    nb = MACRO // P
    arr = np.asarray(res_core, dtype=np.float32).reshape(P, nm, nb, 2)
    return np.ascontiguousarray(
        arr.transpose(1, 2, 0, 3).reshape(RPC, 2)
    )


_NC_CACHE = {}


def get_nc():
    if "nc" not in _NC_CACHE:
        nc = build()
        nc.finalize()
        _NC_CACHE["nc"] = nc
    return _NC_CACHE["nc"]


def kernel(**inputs):
    nc = get_nc()
    in_maps = prepare_in_maps(inputs)
    trace = bool(int(os.environ.get("K_TRACE", "0")))
    try:
        res = run_bass_kernel_spmd(
            nc, in_maps, core_ids=list(range(N_CORES)), trace=trace
        )
    except ModuleNotFoundError:
        res = run_bass_kernel_spmd(nc, in_maps, core_ids=list(range(N_CORES)))
    if res.exec_time_ns is not None:
        print(f"HW exec time: {res.exec_time_ns} ns")
    parts = [unshard_out(res.results[i]["out"]) for i in range(N_CORES)]
    return np.ascontiguousarray(np.concatenate(parts, axis=0))


# revision 16
# speedup vs baseline: 1.2251x; 1.0291x over previous
"""Trainium2 Bass kernel for nn_ActorNetwork (gnn_message_passing).

Pure data-parallel across 8 NeuronCores: each core processes 8192 of the
65536 batch rows; small weights are replicated.

v6: feature-major, neighbor-pair packing; deep software pipeline tuned so
the in-order PE stream never waits on slower engines:
  - state1 uploaded pre-transposed (bf16 [8, 128, rpc]); prefetched two
    macros ahead.
  - state0 + state2 host-packed into one 64-col bf16 block per row
    (56 s2 | 6 s0 | 2 pad); per-neighbor mask sums via a jsum matmul after
    the PE transpose; mask folded into the score PSUM via an identity
    matmul over the -1e30 rows.
  - outputs PE-transposed into a [128, 128] per-core block -> one 512B/part
    store DMA; the out-head matmul + tanh + transpose are pipelined 2-3
    macros behind compute.
  - all small weights packed into two DMAs issued on the Act queue.
  - elementwise work balanced across Act / DVE / Pool.
"""

import os

import numpy as np
import ml_dtypes

import concourse.bass as bass
import concourse.tile as tile
from concourse import bacc
from concourse import mybir
from concourse.bass_utils import run_bass_kernel_spmd

F32 = mybir.dt.float32
F32R = mybir.dt.float32r
BF16 = mybir.dt.bfloat16

N_CORES = 8
B_FULL = 65536
RPC = B_FULL // N_CORES        # rows per core = 8192
MACRO = 512                    # batch rows per macro tile
P = 128

Relu = mybir.ActivationFunctionType.Relu
Tanh = mybir.ActivationFunctionType.Tanh
Exp = mybir.ActivationFunctionType.Exp
Alu = mybir.AluOpType
AX = mybir.AxisListType

# column layout of the packed bf16 weight block [128, WPK_COLS]
_off = 0
def _span(n):
    global _off
    s = (_off, _off + n)
    _off += n
    return s
WG_S = _span(512)        # [128, (8c, 64)]
WS1_S = _span(512)       # [62, (4m, 128)]
W0_S = _span(64)         # [62, 64]
WS2_S = _span(128)       # [128, 128]
WQK_S = _span(128)       # [64, 128]
WC1A_S = _span(128)      # [128, 128]
WVC_S = _span(128)       # [128, 128]
WC2_S = _span(128)       # [128, 128]
WC3_S = _span(2)         # [128, 2]
SEL_S = _span(512)       # [8, (4m, 128)]
SSEL_S = _span(32)       # [128, (4m, 8)]
ID8_S = _span(8)         # [8, 8]
ONES8_S = _span(1)       # [8, 1]
JSUM_S = _span(8)        # [62, 8]
IDB_S = _span(128)       # [128, 128] bf16 identity
WPK_COLS = _off


def build(rpc=RPC, macro=MACRO):
    nm = rpc // macro          # macro tiles per core = 16
    nb = macro // P            # 128-row blocks per macro tile = 4

    nc = bacc.Bacc()

    s1t_d = nc.declare_dram_parameter("s1t", [8, P, rpc], BF16, isOutput=False)
    s2a_d = nc.declare_dram_parameter("s2aug", [P, nm, nb * 64], BF16, isOutput=False)
    wpk_d = nc.declare_dram_parameter("wpk", [P, WPK_COLS], BF16, isOutput=False)
    bpk_d = nc.declare_dram_parameter("bpk", [P, 8], F32, isOutput=False)
    ones18_d = nc.declare_dram_parameter("ones18", [1, 8], F32R, isOutput=False)
    out_d = nc.declare_dram_parameter("out", [P, nm * nb * 2], F32, isOutput=True)

    with tile.TileContext(nc) as tc:
        consts = tc.alloc_tile_pool(name="consts", bufs=1)
        s1T_p = tc.alloc_tile_pool(name="s1T", bufs=3)
        s2_p = tc.alloc_tile_pool(name="s2", bufs=3)
        sm_p = tc.alloc_tile_pool(name="sm", bufs=3)
        work_p = tc.alloc_tile_pool(name="work", bufs=3)
        blk_p = tc.alloc_tile_pool(name="blk", bufs=3)
        psT = tc.alloc_tile_pool(name="psT", bufs=1, space="PSUM")
        psA = tc.alloc_tile_pool(name="psA", bufs=3, space="PSUM")
        psB = tc.alloc_tile_pool(name="psB", bufs=1, space="PSUM")
        psB2 = tc.alloc_tile_pool(name="psB2", bufs=1, space="PSUM")
        psOT = tc.alloc_tile_pool(name="psOT", bufs=1, space="PSUM")
        psC = tc.alloc_tile_pool(name="psC", bufs=1, space="PSUM")

        # ---- packed constants (Act queue; SP starts on state immediately) --
        wpk = consts.tile([P, WPK_COLS], BF16)
        nc.scalar.dma_start(wpk, wpk_d[:, :])
        bpk = consts.tile([P, 8], F32)
        nc.scalar.dma_start(bpk, bpk_d[:, :])
        ones18_sb = consts.tile([1, 8], F32R)
        nc.scalar.dma_start(ones18_sb, ones18_d[:, :])

        def W(span, rows=P):
            return wpk[0:rows, span[0] : span[1]]

        wg_sb = W(WG_S).rearrange("p (c m) -> p c m", c=8)
        ws1_sb = W(WS1_S, 62).rearrange("p (m k) -> p m k", m=4)
        w0_sb = W(W0_S, 62)
        ws2_sb = W(WS2_S)
        wqk_sb = W(WQK_S, 64)
        wc1a_sb = W(WC1A_S)
        wvc_sb = W(WVC_S)
        wc2_sb = W(WC2_S)
        wc3_sb = W(WC3_S)
        sel_sb = W(SEL_S, 8).rearrange("p (m k) -> p m k", m=4)
        ssel_sb = W(SSEL_S).rearrange("p (m k) -> p m k", m=4)
        ident8_sb = W(ID8_S, 8)
        ones8_sb = W(ONES8_S, 8)
        jsum_sb = W(JSUM_S, 62)
        ident_b = W(IDB_S)

        b0bg_sb = bpk[:, 0:1]
        bs1_sb = bpk[:, 1:2]
        bs2_sb = bpk[:, 2:3]
        bc1_sb = bpk[:, 3:4]
        bc2_sb = bpk[:, 4:5]
        bc3_sb = bpk[0:2, 5:6]
        ident2_f = bpk[0:2, 6:8]

        out128_sb = consts.tile([P, nm, nb, 2], F32)

        def emit_F1(ms):
            """State loads for macro ms (prefetched two macros ahead)."""
            row0 = ms * macro
            s1T = s1T_p.tile([P, 8, macro], BF16, tag="s1T")
            nc.sync.dma_start(
                s1T, s1t_d[:, :, row0 : row0 + macro].rearrange("c p b -> p c b")
            )
            s2a = s2_p.tile([P, nb, 64], BF16, tag="s2a")
            nc.sync.dma_start(
                s2a.rearrange("p o k -> p (o k)"), s2a_d[:, ms, :]
            )
            return dict(row0=row0, s1T=s1T, s2a=s2a)

        def emit_OUT(st):
            """Output head + tanh (for macro m-2)."""
            o_ps = psB.tile([2, macro], F32, tag="psB")
            nc.tensor.matmul(o_ps, wc3_sb, st["h2_sb"], start=True, stop=True)
            o_sb = sm_p.tile([2, macro], F32, tag="osb")
            nc.scalar.activation(o_sb, o_ps, Tanh, bias=bc3_sb)
            st["o_sb"] = o_sb

        def emit_B1a(st):
            """Scores (incl. mask) -> exp."""
            sc_ps = psC.tile([8, macro], F32, tag="psC")
            for m in range(4):
                nc.tensor.matmul(
                    sc_ps, ssel_sb[:, m, :], st["qk_sb"][:, m, :],
                    start=(m == 0), stop=False,
                )
            nc.tensor.matmul(sc_ps, ident8_sb, st["nm_sb"], start=False, stop=True)
            p8_sb = sm_p.tile([8, macro], BF16, tag="p8")
            nc.scalar.activation(p8_sb, sc_ps, Exp, scale=0.125)
            st["p8_sb"] = p8_sb

        def emit_F1t(st):
            """state2 transposes (need only the s2a DMA)."""
            s2T_ps = psT.tile([64, nb, P], BF16, tag="s2T")
            for o in range(nb):
                nc.tensor.transpose(s2T_ps[:, o, :], st["s2a"][:, o, :], ident_b)
            st["s2T_ps"] = s2T_ps

        def emit_F2a(st):
            """env matmuls (only need s1T)."""
            eo_ps = psB.tile([P, macro], F32, tag="psB")
            for c in range(8):
                nc.tensor.matmul(
                    eo_ps[64:128, :], wg_sb[:, c], st["s1T"][:, c],
                    start=(c == 0), stop=(c == 7), tile_position=(0, 64),
                )
            st["eo_ps"] = eo_ps

        def emit_B1b1(st):
            """Softmax denominator."""
            den_ps = psC.tile([1, macro], F32, tag="psC")
            nc.tensor.matmul(den_ps, ones8_sb, st["p8_sb"], start=True, stop=True)
            rs_sb = sm_p.tile([1, macro], F32R, tag="rs")
            with nc.allow_low_precision(reason="f32r reciprocal, 19-bit ok"):
                nc.vector.reciprocal(rs_sb, den_ps)
            st["rs_sb"] = rs_sb

        def emit_F2b(st):
            """s2T -> SBUF; mask rows; own matmul -> concatA -> q2."""
            s2T_sb = sm_p.tile([64, nb, P], BF16, tag="s2Tsb")
            nc.scalar.copy(s2T_sb, st["s2T_ps"])
            s2T_flat = s2T_sb[0:62].rearrange("p o k -> p (o k)")
            st["s2T_flat"] = s2T_flat

            nmsum_ps = psC.tile([8, macro], F32, tag="psC")
            nc.tensor.matmul(nmsum_ps, jsum_sb, s2T_flat, start=True, stop=True)
            nm_sb = sm_p.tile([8, macro], BF16, tag="nm")
            nc.gpsimd.tensor_scalar(
                nm_sb, nmsum_ps, 0.0, -1e30, Alu.is_equal, Alu.mult
            )
            st["nm_sb"] = nm_sb

            eo_ps = st["eo_ps"]
            nc.tensor.matmul(eo_ps[0:64, :], w0_sb, s2T_flat, start=True, stop=True)
            concatA = work_p.tile([P, macro], BF16, tag="concatA")
            nc.scalar.activation(concatA, eo_ps, Relu, bias=b0bg_sb)

            q2_ps = psB.tile([P, macro], F32, tag="psB")
            nc.tensor.matmul(q2_ps, wqk_sb, concatA[0:64, :], start=True, stop=True)
            q2_sb = work_p.tile([P, macro], BF16, tag="q2")
            nc.scalar.copy(q2_sb, q2_ps)
            st["concatA"] = concatA
            st["q2_sb"] = q2_sb

        def emit_B1b2(st):
            """1/den broadcast -> alpha."""
            dup_ps = psC.tile([8, macro], F32, tag="psC")
            nc.tensor.matmul(dup_ps, ones18_sb, st["rs_sb"], start=True, stop=True)
            alpha_sb = sm_p.tile([8, macro], BF16, tag="alpha")
            nc.vector.tensor_tensor(alpha_sb, st["p8_sb"], dup_ps, Alu.mult)
            st["alpha_sb"] = alpha_sb

        def emit_F3a(st):
            """i1."""
            i1_sb = blk_p.tile([P, 4, macro], BF16, tag="i1")
            for m in range(4):
                i1_ps = psA.tile([P, macro], F32, tag="psA")
                nc.tensor.matmul(
                    i1_ps, ws1_sb[:, m], st["s2T_flat"], start=True, stop=True
                )
                dst = i1_sb[:, m, :]
                if m == 0:
                    nc.scalar.activation(dst, i1_ps, Relu, bias=bs1_sb)
                elif m == 1:
                    nc.vector.tensor_scalar(dst, i1_ps, bs1_sb, 0.0, Alu.add, Alu.max)
                else:
                    nc.gpsimd.tensor_scalar(dst, i1_ps, bs1_sb, 0.0, Alu.add, Alu.max)
            st["i1_sb"] = i1_sb

        def emit_B2a(st):
            """Weighted i2 -> h1."""
            i2_sb = st["i2_sb"]; alpha_sb = st["alpha_sb"]

            cmul_sb = blk_p.tile([P, 4, macro], BF16, tag="cmul")
            abc = []
            for m in range(4):
                abc_ps = psA.tile([P, macro], F32, tag="psA")
                nc.tensor.matmul(abc_ps, sel_sb[:, m], alpha_sb, start=True, stop=True)
                abc.append(abc_ps)
            for m in range(4):
                if m == 0:
                    nc.vector.tensor_tensor(
                        cmul_sb[:, m, :], i2_sb[:, m, :], abc[m], Alu.mult
                    )
                else:
                    nc.gpsimd.tensor_tensor(
                        cmul_sb[:, m, :], i2_sb[:, m, :], abc[m], Alu.mult
                    )

            h1_ps = psB2.tile([P, macro], F32, tag="psB2")
            nc.tensor.matmul(h1_ps, wc1a_sb, st["concatA"], start=True, stop=False)
            for m in range(4):
                nc.tensor.matmul(
                    h1_ps, wvc_sb, cmul_sb[:, m, :],
                    start=False, stop=(m == 3),
                )
            h1_sb = work_p.tile([P, macro], BF16, tag="h1")
            nc.scalar.activation(h1_sb, h1_ps, Relu, bias=bc1_sb)
            st["h1_sb"] = h1_sb

        def emit_F3b(st):
            """i2, qk."""
            i1_sb = st["i1_sb"]; q2_sb = st["q2_sb"]
            i2_sb = blk_p.tile([P, 4, macro], BF16, tag="i2")
            for m in range(4):
                i2_ps = psA.tile([P, macro], F32, tag="psA")
                nc.tensor.matmul(i2_ps, ws2_sb, i1_sb[:, m, :], start=True, stop=True)
                dst = i2_sb[:, m, :]
                if m == 0:
                    nc.scalar.activation(dst, i2_ps, Relu, bias=bs2_sb)
                elif m == 1:
                    nc.vector.tensor_scalar(dst, i2_ps, bs2_sb, 0.0, Alu.add, Alu.max)
                else:
                    nc.gpsimd.tensor_scalar(dst, i2_ps, bs2_sb, 0.0, Alu.add, Alu.max)

            qk_sb = blk_p.tile([P, 4, macro], BF16, tag="qk")
            nc.vector.tensor_tensor(
                qk_sb, i2_sb,
                q2_sb[:, None, :].to_broadcast((P, 4, macro)),
                Alu.mult,
            )
            st["i2_sb"] = i2_sb
            st["qk_sb"] = qk_sb

        def emit_B3(st):
            """Output transpose into the 128-partition store block."""
            ms = st["row0"] // macro
            oT_ps = psOT.tile([P, nb, 2], F32, tag="oT")
            for o in range(nb):
                nc.tensor.transpose(
                    oT_ps[:, o, :], st["o_sb"][:, o * P : (o + 1) * P], ident2_f
                )
            nc.vector.tensor_copy(out128_sb[:, ms], oT_ps)

        def emit_B2b(st):
            """h2."""
            h2_ps = psB2.tile([P, macro], F32, tag="psB2")
            nc.tensor.matmul(h2_ps, wc2_sb, st["h1_sb"], start=True, stop=True)
            h2_sb = work_p.tile([P, macro], BF16, tag="h2")
            nc.gpsimd.tensor_scalar(h2_sb, h2_ps, bc2_sb, 0.0, Alu.add, Alu.max)
            st["h2_sb"] = h2_sb

        # software pipeline across macros:
        #   iter i computes macro m=i forward phases, m-1 attention/head,
        #   m-2 output head, m-3 output transpose. DMAs prefetch 2 ahead.
        sts = [None] * (nm + 1)
        sts[0] = emit_F1(0)
        sts[1] = emit_F1(1)

        def stage(ms):
            return sts[ms] if 0 <= ms < nm else None

        for ms in range(nm + 3):
            cur, p, p2, p3 = stage(ms), stage(ms - 1), stage(ms - 2), stage(ms - 3)
            if p2 is not None:
                emit_OUT(p2)
            if p is not None:
                emit_B1a(p)
            if cur is not None:
                emit_F1t(cur)
            if ms + 2 < nm:
                sts[ms + 2] = emit_F1(ms + 2)
            if cur is not None:
                emit_F2a(cur)
            if p is not None:
                emit_B1b1(p)
            if cur is not None:
                emit_F2b(cur)
            if p is not None:
                emit_B1b2(p)
            if cur is not None:
                emit_F3a(cur)
            if p is not None:
                emit_B2a(p)
            if cur is not None:
                emit_F3b(cur)
            if p3 is not None:
                emit_B3(p3)
            if p is not None:
                emit_B2b(p)

        nc.sync.dma_start(
            out_d[:, :], out128_sb.rearrange("p a o k -> p (a o k)")
        )

        for _pool in (psC, psOT, psB2, psB, psA, psT, blk_p, work_p, sm_p,
                      s2_p, s1T_p, consts):
            _pool.release()

    return nc


def prepare_in_maps(inputs):
    bf = ml_dtypes.bfloat16
    f32 = np.float32
    nm = RPC // MACRO
    nb = MACRO // P

    def a(x, dt=f32):
        return np.ascontiguousarray(np.asarray(x), dtype=dt)

    W0 = a(inputs["W0"]); Wg = a(inputs["Wg"])
    Ws1 = a(inputs["Ws1"]); Ws2 = a(inputs["Ws2"])
    Wq = a(inputs["Wq"]); Wk = a(inputs["Wk"]); Wv = a(inputs["Wv"])
    Wc1 = a(inputs["Wc1"]); Wc2 = a(inputs["Wc2"]); Wc3 = a(inputs["Wc3"])

    wqk = Wq @ Wk.T                                   # [64, 64]
    wvc = Wv @ Wc1[128:192, :]                        # [64, 128]

    wpk = np.zeros((P, WPK_COLS), dtype=f32)

    def put(span, arr):
        arr = np.asarray(arr, dtype=f32)
        wpk[0 : arr.shape[0], span[0] : span[0] + arr.shape[1]] = arr

    # wg: [128, (8c, 64)] with wg[p, c, :] = Wg[128c + p, :]
    put(WG_S, Wg.reshape(8, P, 64).transpose(1, 0, 2).reshape(P, 512))
    # ws1_blk[7n+j, m, 64r+d] = Ws1[j, d] if n == 2m+r (rows 56:62 zero)
    ws1_blk = np.zeros((62, 4, 128), dtype=f32)
    sel = np.zeros((8, 4, 128), dtype=f32)
    for n in range(8):
        m, r = n // 2, n % 2
        ws1_blk[7 * n : 7 * n + 7, m, 64 * r : 64 * r + 64] = Ws1
        sel[n, m, 64 * r : 64 * r + 64] = 1.0
    put(WS1_S, ws1_blk.reshape(62, 512))
    w0_blk = np.zeros((62, 64), dtype=f32)
    w0_blk[56:62, :] = W0
    put(W0_S, w0_blk)
    ws2_blk = np.zeros((128, 128), dtype=f32)
    ws2_blk[0:64, 0:64] = Ws2
    ws2_blk[64:128, 64:128] = Ws2
    put(WS2_S, ws2_blk)
    put(WQK_S, np.concatenate([wqk, wqk], axis=1))
    put(WC1A_S, Wc1[0:128, :])
    put(WVC_S, np.concatenate([wvc, wvc], axis=0))
    put(WC2_S, Wc2)
    put(WC3_S, Wc3)
    put(SEL_S, sel.reshape(8, 512))
    scoresel = np.zeros((128, 4, 8), dtype=f32)
    for n in range(8):
        m, r = n // 2, n % 2
        scoresel[64 * r : 64 * r + 64, m, n] = 1.0
    put(SSEL_S, scoresel.reshape(128, 32))
    put(ID8_S, np.eye(8, dtype=f32))
    put(ONES8_S, np.ones((8, 1), dtype=f32))
    jsum = np.zeros((62, 8), dtype=f32)
    for n in range(8):
        jsum[7 * n : 7 * n + 7, n] = 1.0
    put(JSUM_S, jsum)
    put(IDB_S, np.eye(128, dtype=f32))

    def col(x):
        return np.ascontiguousarray(np.asarray(x, dtype=f32).reshape(-1, 1))

    bpk = np.zeros((P, 8), dtype=f32)
    bpk[:, 0:1] = np.concatenate([col(inputs["b0"]), col(inputs["bg"])], axis=0)
    bpk[:, 1:2] = np.concatenate([col(inputs["bs1"])] * 2, axis=0)
    bpk[:, 2:3] = np.concatenate([col(inputs["bs2"])] * 2, axis=0)
    bpk[:, 3:4] = col(inputs["bc1"])
    bpk[:, 4:5] = col(inputs["bc2"])
    bpk[0:2, 5:6] = col(inputs["bc3"])
    bpk[0:2, 6:8] = np.eye(2, dtype=f32)

    state0 = a(inputs["state0"]); state1 = a(inputs["state1"])
    state2 = a(inputs["state2"])

    # state1 pre-transposed: s1t[c, p, b] = state1[b, 128c + p], bf16
    s1t_full = np.ascontiguousarray(
        state1.T.reshape(8, P, B_FULL).astype(bf)
    )

    # s2aug: per row 64 cols = 56 s2 | 6 s0 | 2 pad
    s2aug = np.zeros((B_FULL, 64), dtype=bf)
    s2aug[:, 0:56] = state2.reshape(B_FULL, 56).astype(bf)
    s2aug[:, 56:62] = state0.astype(bf)
    # [core, ms, o, p, 64] -> [core, p, ms, (o 64)]
    s2aug_blk = np.ascontiguousarray(
        s2aug.reshape(N_CORES, nm, nb, P, 64).transpose(0, 3, 1, 2, 4)
        .reshape(N_CORES, P, nm, nb * 64)
    )

    shared = {
        "wpk": a(wpk, bf),
        "bpk": bpk,
        "ones18": np.ones((1, 8), dtype=f32),
    }
    in_maps = []
    for i in range(N_CORES):
        m = dict(shared)
        m["s1t"] = np.ascontiguousarray(s1t_full[:, :, i * RPC : (i + 1) * RPC])
        m["s2aug"] = s2aug_blk[i]
        in_maps.append(m)
    return in_maps


def unshard_out(res_core):
    """[128, nm*nb*2] f32 -> [rpc, 2]; out128[p, ms, o, a] = row ms*512+o*128+p."""
    nm = RPC // MACRO
    nb = MACRO // P
    arr = np.asarray(res_core, dtype=np.float32).reshape(P, nm, nb, 2)
    return np.ascontiguousarray(
        arr.transpose(1, 2, 0, 3).reshape(RPC, 2)
    )


_NC_CACHE = {}


def get_nc():
    if "nc" not in _NC_CACHE:
        nc = build()
        nc.finalize()
        _NC_CACHE["nc"] = nc
    return _NC_CACHE["nc"]


def kernel(**inputs):
    nc = get_nc()
    in_maps = prepare_in_maps(inputs)
    trace = bool(int(os.environ.get("K_TRACE", "0")))
    try:
        res = run_bass_kernel_spmd(
            nc, in_maps, core_ids=list(range(N_CORES)), trace=trace
        )
    except ModuleNotFoundError:
        res = run_bass_kernel_spmd(nc, in_maps, core_ids=list(range(N_CORES)))
    if res.exec_time_ns is not None:
        print(f"HW exec time: {res.exec_time_ns} ns")
    parts = [unshard_out(res.results[i]["out"]) for i in range(N_CORES)]
    return np.ascontiguousarray(np.concatenate(parts, axis=0))


# revision 65
# speedup vs baseline: 1.7128x; 1.3981x over previous
"""Trainium2 Bass kernel for nn_ActorNetwork (gnn_message_passing).

Pure data-parallel across 8 NeuronCores: each core processes 8192 of the
65536 batch rows; small weights are replicated.

v6: feature-major, neighbor-pair packing; deep software pipeline tuned so
the in-order PE stream never waits on slower engines:
  - state1 uploaded pre-transposed (bf16 [8, 128, rpc]); prefetched two
    macros ahead.
  - state0 + state2 host-packed into one 64-col bf16 block per row
    (56 s2 | 6 s0 | 2 pad); per-neighbor mask sums via a jsum matmul after
    the PE transpose; mask folded into the score PSUM via an identity
    matmul over the -1e30 rows.
  - outputs PE-transposed into a [128, 128] per-core block -> one 512B/part
    store DMA; the out-head matmul + tanh + transpose are pipelined 2-3
    macros behind compute.
  - all small weights packed into two DMAs issued on the Act queue.
  - elementwise work balanced across Act / DVE / Pool.
"""

import os

import numpy as np
import ml_dtypes

import concourse.bass as bass
import concourse.tile as tile
from concourse import bacc
from concourse import mybir
from concourse.bass_utils import run_bass_kernel_spmd

F32 = mybir.dt.float32
F32R = mybir.dt.float32r
BF16 = mybir.dt.bfloat16
F8 = mybir.dt.float8e4
DR = mybir.MatmulPerfMode.DoubleRow

N_CORES = 8
B_FULL = 65536
RPC = B_FULL // N_CORES        # rows per core = 8192
MACRO = 512                    # batch rows per macro tile
P = 128

Relu = mybir.ActivationFunctionType.Relu
Tanh = mybir.ActivationFunctionType.Tanh
Exp = mybir.ActivationFunctionType.Exp
Alu = mybir.AluOpType
AX = mybir.AxisListType

# column layout of the packed bf16 weight block [128, WPK_COLS]
_off = 0
def _span(n):
    global _off
    s = (_off, _off + n)
    _off += n
    return s
WG_S = _span(512)        # [128, (8c, 64)]
WS1_S = _span(512)       # [62, (4m, 128)]
WS2_S = _span(128)       # [128, 128]
WQK_S = _span(128)       # [64, 128]
WC1A_S = _span(128)      # [128, 128]
WVC_S = _span(128)       # [128, 128]
WC2_S = _span(128)       # [128, 128]
WC3_S = _span(2)         # [128, 2]
SEL_S = _span(512)       # [8, (4m, 128)]
SSEL_S = _span(32)       # [128, (4m, 8)]
ONES8_S = _span(1)       # [8, 1]
ID8_S = _span(8)         # [8, 8]
JOWN_S = _span(64)       # [62, 64] w0 rows 56:62
IDB_S = _span(128)       # [128, 128] bf16 identity
WPK_COLS = _off


def build(rpc=RPC, macro=MACRO):
    nm = rpc // macro          # macro tiles per core = 16
    nb = macro // P            # 128-row blocks per macro tile = 4

    nc = bacc.Bacc()

    s1t_d = nc.declare_dram_parameter("s1t8", [2, 8, P, rpc], F8, isOutput=False)
    s2a_d = nc.declare_dram_parameter("s2aug", [P, nm, nb * 128], BF16, isOutput=False)
    wpk_d = nc.declare_dram_parameter("wpk", [P, WPK_COLS], BF16, isOutput=False)
    w8pk_d = nc.declare_dram_parameter("w8pk", [P, 1024], F8, isOutput=False)
    bpk_d = nc.declare_dram_parameter("bpk", [P, 10], F32, isOutput=False)
    out_d = nc.declare_dram_parameter("out", [P, nm * nb * 2], F32, isOutput=True)

    from concourse import library_config

    with tile.TileContext(nc) as tc:
        nc.gpsimd.load_library(library_config.proxy)
        consts = tc.alloc_tile_pool(name="consts", bufs=1)
        s1T_p = tc.alloc_tile_pool(name="s1T", bufs=3)
        s2_p = tc.alloc_tile_pool(name="s2", bufs=3)
        sm_p = tc.alloc_tile_pool(name="sm", bufs=4)
        work_p = tc.alloc_tile_pool(name="work", bufs=4)
        blk_p = tc.alloc_tile_pool(name="blk", bufs=4)
        psA = tc.alloc_tile_pool(name="psA", bufs=5, space="PSUM")
        psB = tc.alloc_tile_pool(name="psB", bufs=1, space="PSUM")
        psB2 = tc.alloc_tile_pool(name="psB2", bufs=1, space="PSUM")
        psC = tc.alloc_tile_pool(name="psC", bufs=1, space="PSUM")

        # ---- packed constants (Act queue; SP starts on state immediately) --
        wpk = consts.tile([P, WPK_COLS], BF16)
        nc.scalar.dma_start(wpk, wpk_d[:, :])
        w8pk = consts.tile([P, 2, 4, 2, 64], F8)
        nc.scalar.dma_start(
            w8pk.rearrange("p h c t m -> p (h c t m)"), w8pk_d[:, :]
        )
        bpk = consts.tile([P, 10], F32)
        nc.scalar.dma_start(bpk, bpk_d[:, :])

        def W(span, rows=P):
            return wpk[0:rows, span[0] : span[1]]

        wg_sb = W(WG_S).rearrange("p (c m) -> p c m", c=8)
        ws1_sb = W(WS1_S, 62).rearrange("p (m k) -> p m k", m=4)
        ws2_sb = W(WS2_S)
        wqk_sb = W(WQK_S, 64)
        wc1a_sb = W(WC1A_S)
        wvc_sb = W(WVC_S)
        wc2_sb = W(WC2_S)
        wc3_sb = W(WC3_S)
        sel_sb = W(SEL_S, 8).rearrange("p (m k) -> p m k", m=4)
        ssel_sb = W(SSEL_S).rearrange("p (m k) -> p m k", m=4)
        ones8_sb = W(ONES8_S, 8)
        ident8_sb = W(ID8_S, 8)
        jown_sb = W(JOWN_S, 62)
        ident_b = W(IDB_S)

        b0bg_sb = bpk[:, 0:1]
        bs1_sb = bpk[:, 1:2]
        bs2_sb = bpk[:, 2:3]
        bc1_sb = bpk[:, 3:4]
        bc2_sb = bpk[:, 4:5]
        bc3_sb = bpk[0:2, 5:6]
        ident2_f = bpk[0:2, 6:8]

        out128_sb = consts.tile([P, nm, nb, 2], F32)

        def emit_F1(ms):
            """State loads for macro ms (prefetched two macros ahead)."""
            row0 = ms * macro
            s2a = s2_p.tile([P, nb, 128], BF16, tag="s2a")
            nc.sync.dma_start(
                s2a.rearrange("p o k -> p (o k)"), s2a_d[:, ms, :]
            )
            s1T = s1T_p.tile([P, 2, 8, macro], F8, tag="s1T")
            nc.sync.dma_start(
                s1T,
                s1t_d[:, :, :, row0 : row0 + macro].rearrange(
                    "h c p b -> p h c b"
                ),
            )
            return dict(row0=row0, s1T=s1T, s2a=s2a)

        def emit_OUT(st):
            """Output head + tanh (for macro m-2)."""
            o_ps = psB.tile([2, macro], F32, tag="psB")
            nc.tensor.matmul(o_ps, wc3_sb, st["h2_sb"], start=True, stop=True)
            o_sb = sm_p.tile([2, macro], F32, tag="osb")
            nc.scalar.activation(o_sb, o_ps, Tanh, bias=bc3_sb)
            st["o_sb"] = o_sb

        def emit_B1a(st):
            """Scores (incl. mask) -> exp."""
            sc_ps = psC.tile([8, macro], F32, tag="psC")
            for m in range(4):
                nc.tensor.matmul(
                    sc_ps, ssel_sb[:, m, :], st["qk_sb"][:, m, :],
                    start=(m == 0), stop=False,
                )
            nc.tensor.matmul(sc_ps, ident8_sb, st["nm_sb"], start=False, stop=True)
            p8_sb = sm_p.tile([8, macro], BF16, tag="p8")
            nc.scalar.activation(p8_sb, sc_ps, Exp, scale=0.125)
            st["p8_sb"] = p8_sb

        def emit_F1t(st):
            """mask sums (batch-major) + xbar DMA transposes straight to SBUF."""
            s2a = st["s2a"]
            with nc.allow_low_precision(reason="mask sums only compared to 0"):
                nc.vector.tensor_reduce(
                    s2a[:, :, 64:72],
                    s2a[:, :, 0:56].rearrange("p o (n j) -> p o n j", j=7),
                    AX.X,
                    Alu.add,
                )
            s2T_sb = sm_p.tile([P, nb, P], BF16, tag="s2Tsb")
            for o in range(nb):
                nc.sync.dma_start_transpose(s2T_sb[:, o, :], s2a[:, o, :])
            st["s2T_sb"] = s2T_sb

        def emit_F2a(st):
            """env matmuls: fp8 DoubleRow, hi@Wa + hi@Wb + lo@Wa."""
            env_ps = psB.tile([64, macro], F32, tag="psB")
            s1T = st["s1T"]
            first = True
            for h, w in ((0, 0), (0, 1), (1, 0)):
                for c4 in range(4):
                    nc.tensor.matmul(
                        env_ps,
                        w8pk[:, w, c4],
                        s1T[:, h, 2 * c4 : 2 * c4 + 2, :],
                        start=first, stop=(h, w, c4) == (1, 0, 3),
                        perf_mode=DR,
                    )
                    first = False
            st["env_ps"] = env_ps

        def emit_B1b1(st):
            """Softmax denominator."""
            den_ps = psC.tile([1, macro], F32, tag="psC")
            nc.tensor.matmul(den_ps, ones8_sb, st["p8_sb"], start=True, stop=True)
            rs_sb = sm_p.tile([1, macro], F32R, tag="rs")
            with nc.allow_low_precision(reason="f32r reciprocal, 19-bit ok"):
                nc.vector.reciprocal(rs_sb, den_ps)
            st["rs_sb"] = rs_sb

        def emit_F2b(st):
            """own matmul; mask rows; concatA; q2."""
            s2T_sb = st["s2T_sb"]
            s2T_flat = s2T_sb[0:62].rearrange("p o k -> p (o k)")
            st["s2T_flat"] = s2T_flat

            nm_sb = sm_p.tile([8, macro], BF16, tag="nm")
            nc.vector.tensor_scalar(
                nm_sb,
                s2T_sb[64:72].rearrange("p o k -> p (o k)"),
                0.0, -1e30, Alu.is_equal, Alu.mult,
            )
            st["nm_sb"] = nm_sb

            jo_ps = psA.tile([64, macro], F32, tag="psA")
            nc.tensor.matmul(jo_ps, jown_sb, s2T_flat, start=True, stop=True)
            concatA = work_p.tile([P, macro], BF16, tag="concatA")
            nc.scalar.activation(
                concatA[0:64, :], jo_ps[0:64], Relu, bias=b0bg_sb[0:64]
            )
            nc.scalar.activation(
                concatA[64:128, :], st["env_ps"], Relu,
                bias=b0bg_sb[64:128], scale=1.0 / 32.0,
            )

            q2_ps = psB.tile([P, macro], F32, tag="psB")
            nc.tensor.matmul(q2_ps, wqk_sb, concatA[0:64, :], start=True, stop=True)
            q2_sb = work_p.tile([P, macro], BF16, tag="q2")
            nc.scalar.copy(q2_sb, q2_ps)
            st["concatA"] = concatA
            st["q2_sb"] = q2_sb

        def emit_B1b2(st):
            """1/den broadcast -> alpha."""
            rdup_sb = sm_p.tile([8, macro], F32R, tag="rdup")
            nc.gpsimd.partition_broadcast(rdup_sb, st["rs_sb"], channels=8)
            alpha_sb = sm_p.tile([8, macro], BF16, tag="alpha")
            nc.gpsimd.tensor_tensor(alpha_sb, st["p8_sb"], rdup_sb, Alu.mult)
            st["alpha_sb"] = alpha_sb

        def emit_F3a(st):
            """i1."""
            i1_sb = blk_p.tile([P, 4, macro], BF16, tag="i1")
            for m in range(4):
                i1_ps = psA.tile([P, macro], F32, tag="psA")
                nc.tensor.matmul(
                    i1_ps, ws1_sb[:, m], st["s2T_flat"], start=True, stop=True
                )
                dst = i1_sb[:, m, :]
                if m in (0, 2):
                    nc.scalar.activation(dst, i1_ps, Relu, bias=bs1_sb)
                else:
                    nc.vector.tensor_scalar(dst, i1_ps, bs1_sb, 0.0, Alu.add, Alu.max)
            st["i1_sb"] = i1_sb

        def emit_B2a(st):
            """Weighted i2 -> h1."""
            i2_sb = st["i2_sb"]; alpha_sb = st["alpha_sb"]

            cmul_sb = blk_p.tile([P, 4, macro], BF16, tag="cmul")
            abc = []
            for m in range(4):
                abc_ps = psA.tile([P, macro], F32, tag="psA")
                nc.tensor.matmul(abc_ps, sel_sb[:, m], alpha_sb, start=True, stop=True)
                abc.append(abc_ps)
            for m in range(4):
                nc.vector.tensor_tensor(
                    cmul_sb[:, m, :], i2_sb[:, m, :], abc[m], Alu.mult
                )

            h1_ps = psB2.tile([P, macro], F32, tag="psB2")
            nc.tensor.matmul(h1_ps, wc1a_sb, st["concatA"], start=True, stop=False)
            for m in range(4):
                nc.tensor.matmul(
                    h1_ps, wvc_sb, cmul_sb[:, m, :],
                    start=False, stop=(m == 3),
                )
            h1_sb = work_p.tile([P, macro], BF16, tag="h1")
            nc.scalar.activation(h1_sb, h1_ps, Relu, bias=bc1_sb)
            st["h1_sb"] = h1_sb

        def emit_F3b(st):
            """i2, qk."""
            i1_sb = st["i1_sb"]; q2_sb = st["q2_sb"]
            i2_sb = blk_p.tile([P, 4, macro], BF16, tag="i2")
            for m in range(4):
                i2_ps = psA.tile([P, macro], F32, tag="psA")
                nc.tensor.matmul(i2_ps, ws2_sb, i1_sb[:, m, :], start=True, stop=True)
                dst = i2_sb[:, m, :]
                if m in (0, 2):
                    nc.scalar.activation(dst, i2_ps, Relu, bias=bs2_sb)
                else:
                    nc.vector.tensor_scalar(dst, i2_ps, bs2_sb, 0.0, Alu.add, Alu.max)

            qk_sb = blk_p.tile([P, 4, macro], BF16, tag="qk")
            nc.gpsimd.tensor_tensor(
                qk_sb[:, 0:2, :], i2_sb[:, 0:2, :],
                q2_sb[:, None, :].to_broadcast((P, 2, macro)),
                Alu.mult,
            )
            nc.gpsimd.tensor_tensor(
                qk_sb[:, 2:4, :], i2_sb[:, 2:4, :],
                q2_sb[:, None, :].to_broadcast((P, 2, macro)),
                Alu.mult,
            )
            st["i2_sb"] = i2_sb
            st["qk_sb"] = qk_sb

        def emit_B3(st):
            """Output transpose into the 128-partition store block."""
            ms = st["row0"] // macro
            oT_ps = psC.tile([P, nb, 2], F32, tag="psC")
            for o in range(nb):
                nc.tensor.transpose(
                    oT_ps[:, o, :], st["o_sb"][:, o * P : (o + 1) * P], ident2_f
                )
            nc.vector.tensor_copy(out128_sb[:, ms], oT_ps)

        def emit_B2b(st):
            """h2."""
            h2_ps = psB2.tile([P, macro], F32, tag="psB2")
            nc.tensor.matmul(h2_ps, wc2_sb, st["h1_sb"], start=True, stop=True)
            h2_sb = work_p.tile([P, macro], BF16, tag="h2")
            nc.scalar.activation(h2_sb, h2_ps, Relu, bias=bc2_sb)
            st["h2_sb"] = h2_sb

        # software pipeline across macros:
        #   iter i: forward phases for macro i, softmax (B1) for i-1,
        #   weighted-sum/head (B2) for i-2, output head for i-3, output
        #   transpose for i-4. Each chain segment gets a full iteration of
        #   slack. DMAs prefetch 2 ahead.
        sts = [None] * (nm + 1)
        sts[0] = emit_F1(0)
        sts[1] = emit_F1(1)

        def stage(ms):
            return sts[ms] if 0 <= ms < nm else None

        for ms in range(nm + 5):
            cur = stage(ms)
            p1, p2, p3, p4 = (
                stage(ms - 1), stage(ms - 2), stage(ms - 3), stage(ms - 4)
            )
            if p1 is not None:
                emit_B1a(p1)
            if cur is not None:
                emit_F1t(cur)
            if ms + 2 < nm:
                sts[ms + 2] = emit_F1(ms + 2)
            if cur is not None:
                emit_F2a(cur)
            if p1 is not None:
                emit_B1b1(p1)
            if cur is not None:
                emit_F2b(cur)
            if p3 is not None:
                emit_OUT(p3)
            if p1 is not None:
                emit_B1b2(p1)
            if cur is not None:
                emit_F3a(cur)
            if p2 is not None:
                emit_B2a(p2)
            if cur is not None:
                emit_F3b(cur)
            if p4 is not None:
                emit_B3(p4)
            if p2 is not None:
                emit_B2b(p2)

        nc.sync.dma_start(
            out_d[:, :], out128_sb.rearrange("p a o k -> p (a o k)")
        )

        for _pool in (psC, psB2, psB, psA, blk_p, work_p, sm_p,
                      s2_p, s1T_p, consts):
            _pool.release()

    return nc


def prepare_in_maps(inputs):
    bf = ml_dtypes.bfloat16
    f32 = np.float32
    nm = RPC // MACRO
    nb = MACRO // P

    def a(x, dt=f32):
        return np.ascontiguousarray(np.asarray(x), dtype=dt)

    W0 = a(inputs["W0"]); Wg = a(inputs["Wg"])
    Ws1 = a(inputs["Ws1"]); Ws2 = a(inputs["Ws2"])
    Wq = a(inputs["Wq"]); Wk = a(inputs["Wk"]); Wv = a(inputs["Wv"])
    Wc1 = a(inputs["Wc1"]); Wc2 = a(inputs["Wc2"]); Wc3 = a(inputs["Wc3"])

    f8 = ml_dtypes.float8_e4m3fn
    wqk = Wq @ Wk.T                                   # [64, 64]
    wvc = Wv @ Wc1[128:192, :]                        # [64, 128]

    wpk = np.zeros((P, WPK_COLS), dtype=f32)

    def put(span, arr):
        arr = np.asarray(arr, dtype=f32)
        wpk[0 : arr.shape[0], span[0] : span[0] + arr.shape[1]] = arr

    # wg fp8 hi/lo split, scaled by 32 into e4m3's normal range; layout
    # [p, (hi/lo, c4, t, 64)] with chunk index c = 2*c4 + t
    wg32 = Wg * 32.0
    wg_hi = wg32.astype(f8)
    wg_lo = (wg32 - wg_hi.astype(f32)).astype(f8)
    w8pk = np.stack(
        [
            w.reshape(4, 2, P, 64).transpose(2, 0, 1, 3).reshape(P, 512)
            for w in (wg_hi, wg_lo)
        ],
        axis=1,
    ).reshape(P, 1024)
    # ws1_blk[7n+j, m, 64r+d] = Ws1[j, d] if n == 2m+r (rows 56:62 zero)
    ws1_blk = np.zeros((62, 4, 128), dtype=f32)
    sel = np.zeros((8, 4, 128), dtype=f32)
    for n in range(8):
        m, r = n // 2, n % 2
        ws1_blk[7 * n : 7 * n + 7, m, 64 * r : 64 * r + 64] = Ws1
        sel[n, m, 64 * r : 64 * r + 64] = 1.0
    put(WS1_S, ws1_blk.reshape(62, 512))
    jown = np.zeros((62, 64), dtype=f32)
    jown[56:62, :] = W0
    put(JOWN_S, jown)
    ws2_blk = np.zeros((128, 128), dtype=f32)
    ws2_blk[0:64, 0:64] = Ws2
    ws2_blk[64:128, 64:128] = Ws2
    put(WS2_S, ws2_blk)
    put(WQK_S, np.concatenate([wqk, wqk], axis=1))
    put(WC1A_S, Wc1[0:128, :])
    put(WVC_S, np.concatenate([wvc, wvc], axis=0))
    put(WC2_S, Wc2)
    put(WC3_S, Wc3)
    put(SEL_S, sel.reshape(8, 512))
    scoresel = np.zeros((128, 4, 8), dtype=f32)
    for n in range(8):
        m, r = n // 2, n % 2
        scoresel[64 * r : 64 * r + 64, m, n] = 1.0
    put(SSEL_S, scoresel.reshape(128, 32))
    put(ONES8_S, np.ones((8, 1), dtype=f32))
    put(ID8_S, np.eye(8, dtype=f32))
    put(IDB_S, np.eye(128, dtype=f32))

    def col(x):
        return np.ascontiguousarray(np.asarray(x, dtype=f32).reshape(-1, 1))

    bpk = np.zeros((P, 10), dtype=f32)
    bpk[:, 0:1] = np.concatenate([col(inputs["b0"]), col(inputs["bg"])], axis=0)
    bpk[:, 1:2] = np.concatenate([col(inputs["bs1"])] * 2, axis=0)
    bpk[:, 2:3] = np.concatenate([col(inputs["bs2"])] * 2, axis=0)
    bpk[:, 3:4] = col(inputs["bc1"])
    bpk[:, 4:5] = col(inputs["bc2"])
    bpk[0:2, 5:6] = col(inputs["bc3"])
    bpk[0:2, 6:8] = np.eye(2, dtype=f32)
    bpk[0:64, 8] = 1.0          # own rows: unscaled
    bpk[64:128, 8] = 1.0 / 32.0  # env rows: undo the fp8 weight scaling

    state0 = a(inputs["state0"]); state1 = a(inputs["state1"])
    state2 = a(inputs["state2"])

    # state1 pre-transposed + fp8 hi/lo split: s1t8[h, c, p, b]
    s1t_f = state1.T.reshape(8, P, B_FULL)
    s1_hi = s1t_f.astype(f8)
    s1_lo = (s1t_f - s1_hi.astype(f32)).astype(f8)
    s1t_full = np.ascontiguousarray(np.stack([s1_hi, s1_lo], axis=0))

    # s2aug: per row 128 cols = 56 s2 | 6 s0 | 2 pad | 8 mask slots | pad
    s2aug = np.zeros((B_FULL, 128), dtype=bf)
    s2aug[:, 0:56] = state2.reshape(B_FULL, 56).astype(bf)
    s2aug[:, 56:62] = state0.astype(bf)
    # [core, ms, o, p, 128] -> [core, p, ms, (o 128)]
    s2aug_blk = np.ascontiguousarray(
        s2aug.reshape(N_CORES, nm, nb, P, 128).transpose(0, 3, 1, 2, 4)
        .reshape(N_CORES, P, nm, nb * 128)
    )

    shared = {
        "wpk": a(wpk, bf),
        "w8pk": np.ascontiguousarray(w8pk),
        "bpk": bpk,
    }
    in_maps = []
    for i in range(N_CORES):
        m = dict(shared)
        m["s1t8"] = np.ascontiguousarray(
            s1t_full[:, :, :, i * RPC : (i + 1) * RPC]
        )
        m["s2aug"] = s2aug_blk[i]
        in_maps.append(m)
    return in_maps


def unshard_out(res_core):
    """[128, nm*nb*2] f32 -> [rpc, 2]; out128[p, ms, o, a] = row ms*512+o*128+p."""
    nm = RPC // MACRO
    nb = MACRO // P
    arr = np.asarray(res_core, dtype=np.float32).reshape(P, nm, nb, 2)
    return np.ascontiguousarray(
        arr.transpose(1, 2, 0, 3).reshape(RPC, 2)
    )


_NC_CACHE = {}


def get_nc():
    if "nc" not in _NC_CACHE:
        nc = build()
        nc.finalize()
        _NC_CACHE["nc"] = nc
    return _NC_CACHE["nc"]


def kernel(**inputs):
    nc = get_nc()
    in_maps = prepare_in_maps(inputs)
    trace = bool(int(os.environ.get("K_TRACE", "0")))
    try:
        res = run_bass_kernel_spmd(
            nc, in_maps, core_ids=list(range(N_CORES)), trace=trace
        )
    except ModuleNotFoundError:
        res = run_bass_kernel_spmd(nc, in_maps, core_ids=list(range(N_CORES)))
    if res.exec_time_ns is not None:
        print(f"HW exec time: {res.exec_time_ns} ns")
    parts = [unshard_out(res.results[i]["out"]) for i in range(N_CORES)]
    return np.ascontiguousarray(np.concatenate(parts, axis=0))
